# revision 31
# baseline (speedup 1.0000x reference)
"""Trainium2 Bass kernel for a ReActNet-style binary BasicBlock.

Reference math per block (twice, with different weights):
    s   = sign(x + b_in)                      # +-1
    c   = conv3x3(s, mean|w| * sign(w))       # binarized conv, pad=1
    y   = x + ALPHA * c                       # residual
    y   = prelu(y + b_mid, a) + b_out

Key facts exploited:
  * matmul inputs are exactly +-1 -> bf16 matmuls are EXACT (integer sums
    accumulated in fp32 PSUM).
  * per-output-channel weight scale factors out:  conv(s, scale*sign(w)) =
    scale .* conv(s, sign(w)).
  * residual rides through PSUM via a bf16 matmul with diag(1/(ALPHA*scale))
    (bf16 streams 1 cycle/row on the PE; fp32 costs 4): x and p1 are held /
    evicted as bf16 (~0.4% rounding, well inside the 2e-2 gate).
        T = binconv(s) + x / as           (as = ALPHA*scale, per channel)
    then prelu(x + as*binconv + b, a) = Prelu-activation(T) with
    per-partition scale=as, bias=b, alpha=a  -- a single ScalarE op reading
    PSUM directly.  (prelu positive homogeneity: as > 0.)

Layout (q4i scheme): NCHW, channels (64) on partitions; FOUR images in
flight on the four 64x64 PE tiles, PAIRED into two PSUM banks (diagonal
row-split pairs write complementary partition halves of one bank; the
per-partition has_written zero-regions keep the accumulation groups
independent):
    imgA: tile(0,0)   rows 0-63  -> bankX 0-63    (aligned)
    imgB: tile(64,64) rows 64-127-> bankX 64-127  (aligned)
    imgC: tile(0,64) / tile(64,0) alternating per conv -> bankY (crossed)
    imgD: tile(64,0) / tile(0,64) alternating per conv -> bankY (crossed)
The crossed tiles flip C/D's partition home after each conv; all engine ops
stay partition-aligned, DMA handles the final placement for free.
Bank pairing lets every PSUM eviction run as ONE 128-partition ScalarE
Prelu op per pair (vs two 64-partition ops): half the ACT instructions at
full lane utilization.

Engine balance: signs run on the DVE as single tensor_scalar ops
producing +-0.5 ((x >= -b) - 0.5); the missing x2 is folded into the
per-channel eviction scale (asc = 2*ALPHA*mean|w|), keeping matmuls
exact.  Store DMAs issue from the GpSimd queue, loads from Sync, so the
ScalarE queue does nothing but the two fused Prelu evictions per chunk.

Spatial strips of R=16 output rows.  x is DMA'd into contiguous unpadded
staging (8.9KB descriptor runs, vs 448B for a padded layout) and restrided
to the padded-114 conv layout by the Sign / bf16-cast ops whose in/out APs
differ anyway; p2 is evicted unpadded for the same reason on the store
side.  conv1 rows [h0-1,h0+1) are carried over from the previous strip's
p1 (no halo recompute); s2 is signed in chunk-sized pieces so conv2 can
start before the whole strip is evicted.

Sharding: pure data parallel, batch 32 -> 4 images x 8 cores, weights
replicated, no collectives.
"""

import numpy as np
from contextlib import ExitStack, nullcontext

import concourse.bass as bass
import concourse.tile as tile
from concourse import mybir
from concourse import bacc
from concourse.bass_utils import run_bass_kernel_spmd
from concourse.masks import make_identity

B, C, H, W = 32, 64, 112, 112
ALPHA = 0.25
NCORES = 8
BL = B // NCORES          # images per core
WP = W + 2                # padded width
R = 16                    # output rows per strip
NSTRIPS = H // R

F32 = mybir.dt.float32
BF16 = mybir.dt.bfloat16

WVEC_NAMES = ["b11", "b12", "b13", "b21", "b22", "b23", "a1", "a2"]

SKIP_STORE = False   # timing experiment: drop output DMA
SKIP_LOAD = False    # timing experiment: drop input DMA (garbage data)
RESID_LAST = True    # residual matmul in slot 9 (vs slot 0)
P2_BUFS = 2          # p2 double/triple buffering
S1_DVE = True        # s1 sign on DVE (else ScalarE Sign, +-1 w/ asc1 x1)
S2_DVE = True        # s2 sign on DVE (else ScalarE Sign)
B23_GPSIMD = True    # final +b23 on GpSimd instead of DVE


def _bcast_ap(dram_ap, reps=2):
    """Source AP replicating a DRAM tensor across partition groups."""
    return bass.AP(
        tensor=dram_ap.tensor,
        offset=dram_ap.offset,
        ap=[[0, reps]] + [list(d) for d in dram_ap.ap],
    )


def _row_chunks(lo, hi, step=4):
    r = lo
    while r < hi:
        yield r, min(step, hi - r)
        r += step


def build_program(bl=BL, loop_n=None):
    """Build the Bass program for one core processing `bl` images.

    loop_n: if set, repeat the whole main loop on-device that many times
    (timing harness only -- results identical, just recomputed).
    """
    nc = bacc.Bacc("TRN2", target_bir_lowering=False, debug=False)

    x_d = nc.dram_tensor("x", [bl, C, H, W], F32, kind="ExternalInput").ap()
    w3_d = nc.dram_tensor("w3", [C, C, 3, 3], F32, kind="ExternalInput").ap()
    wpw_d = nc.dram_tensor("w_pw", [C, C, 3, 3], F32, kind="ExternalInput").ap()
    vec_d = {
        n: nc.dram_tensor(n, [C], F32, kind="ExternalInput").ap()
        for n in WVEC_NAMES
    }
    # bf16 output (host casts back to f32): halves the store traffic and
    # stays well inside the 2e-2 gate (adds <= 2^-9 relative rounding)
    out_d = nc.dram_tensor("out", [bl, C, H, W], BF16, kind="ExternalOutput").ap()

    with tile.TileContext(nc) as tc:
        _kernel_body(tc, out_d, x_d, w3_d, wpw_d, vec_d, bl, loop_n=loop_n)

    nc.compile()
    return nc


def _prep_conv_consts(nc, const, wdram, name, samp=1.0):
    """Per-conv constants: binarized-transposed weights, the eviction scale
    asc = ALPHA*mean|w|/samp (samp = sign amplitude: 0.5 when the DVE sign
    produces +-0.5), diag(1/asc) for the residual matmul.  Everything
    replicated on both partition halves."""
    # natural layout [co, ci*3*3] duplicated -> per-channel scale
    wn = const.tile([128, C * 9], F32, name=f"wn_{name}")
    nc.sync.dma_start(out=wn, in_=_bcast_ap(wdram.rearrange("a b c d -> a (b c d)")))
    wabs = const.tile([128, C * 9], F32, name=f"wabs_{name}")
    asum = const.tile([128, 1], F32, name=f"asum_{name}")
    nc.scalar.activation(
        out=wabs, in_=wn, func=mybir.ActivationFunctionType.Abs, accum_out=asum
    )
    asc = const.tile([128, 1], F32, name=f"asc_{name}")
    nc.vector.tensor_scalar_mul(asc, asum, ALPHA / (C * 9) / samp)
    inv_asc = const.tile([128, 1], F32, name=f"iasc_{name}")
    nc.vector.reciprocal(inv_asc, asc)

    # gathered+transposed weights [ci(+dup), co, tap], then binarize to bf16
    wg = const.tile([128, C, 9], F32, name=f"wg_{name}")
    src = bass.AP(
        tensor=wdram.tensor,
        offset=wdram.offset,
        ap=[[9, C], [C * 9, C], [1, 9]],
    )
    for rep in range(2):
        nc.sync.dma_start(out=wg[64 * rep : 64 * rep + 64, :, :], in_=src)
    wsign = const.tile([128, C, 9], BF16, name=f"ws_{name}")
    nc.scalar.activation(out=wsign, in_=wg, func=mybir.ActivationFunctionType.Sign)

    # residual injector: diag(1/as) bf16 (bf16 matmul = 1 cyc/row vs 4 for
    # fp32), per partition half
    ident = const.tile([128, C], BF16, name=f"id_{name}")
    make_identity(nc, ident[0:64, :])
    make_identity(nc, ident[64:128, :])
    nc.vector.tensor_scalar_mul(ident, ident, inv_asc)
    return wsign, asc, ident


LO = slice(0, 64)
HI = slice(64, 128)


def _conv_quad(nc, ps, w, ident, quads, pair_outs, soff_of, nr, asc, bias_mid,
               alpha, strip_pads=False):
    """One double-chunk (up to 8 output rows) of conv for four images on the
    four 64x64 PE tiles.

    quads: list of 4 tuples (rsl, osl, s_t, res_of):
      rsl: SBUF row half this image's data lives on (slice)
      osl: PSUM partition half this image's results land on (slice)
      s_t: sign tile; res_of(rsl, dr, snr): residual rhs AP for a sub-chunk
    Images 0,1 share a PSUM tile ([128, 1024] = 2 banks) and images 2,3 a
    second one (complementary partition halves; per-partition has_written
    keeps the accumulation groups independent, and the diagonal tile pairs
    stream concurrently).  Rows split into two <=4-row sub-chunks, one per
    bank (cols 0:456 and 512:968), so a full 8-row double-chunk evicts as
    ONE 128-partition ACT per pair reading both banks.
    Slots: the 9 taps then the residual-injector matmul.
    Eviction: out = Prelu(asc*psum + bias) per pair.
    """
    npair = (len(quads) + 1) // 2
    pts = [ps.tile([128, 1024], F32, tag="ps", name="pt") for _ in range(npair)]
    subs = [(0, min(4, nr))] + ([(4, nr - 4)] if nr > 4 else [])
    rslot = 9 if RESID_LAST else 0
    for s in range(10):
        for qi, (rsl, osl, s_t, res_of) in enumerate(quads):
            pt = pts[qi // 2]
            for dr, snr in subs:
                nn = snr * WP
                pc = 0 if dr == 0 else 512
                if s == rslot:
                    # residual slot: taps elsewhere need only s_t
                    nc.tensor.matmul(
                        pt[osl, pc : pc + nn], ident[rsl, :],
                        res_of(rsl, dr, snr),
                        start=(s == 0), stop=(s == 9), skip_group_check=True,
                    )
                else:
                    t = s - 1 if s > rslot else s
                    so = soff_of(rsl, t) + dr * WP
                    nc.tensor.matmul(
                        pt[osl, pc : pc + nn], w[rsl, :, t],
                        s_t[rsl, so : so + nn],
                        start=(s == 0), stop=(s == 9), skip_group_check=True,
                    )
    for pt, out_ap in zip(pts, pair_outs):
        # out_ap covers the pair's full nr rows: flat [128, nr*WP] (conv1,
        # padded) or [128, nr, W] (conv2, pads stripped)
        if nr == 8:  # one ACT reading both banks
            if strip_pads:
                src = bass.AP(
                    tensor=pt.tensor, offset=pt.offset + 1,
                    ap=[list(pt.ap[0]), [512, 2], [WP, 4], [1, W]],
                )
            else:
                src = bass.AP(
                    tensor=pt.tensor, offset=pt.offset,
                    ap=[list(pt.ap[0]), [512, 2], [1, 456]],
                )
            nc.scalar.activation(
                out=out_ap, in_=src,
                func=mybir.ActivationFunctionType.Prelu,
                bias=bias_mid, scale=asc, alpha=alpha,
            )
        else:
            for dr, snr in subs:
                pc = 0 if dr == 0 else 512
                src = pt[:, pc : pc + snr * WP]
                if strip_pads:
                    src = src.rearrange("p (r c) -> p r c", c=WP)[:, :, 1 : 1 + W]
                    dst = out_ap[:, dr : dr + snr, :]
                else:
                    dst = out_ap[:, dr * WP : (dr + snr) * WP]
                nc.scalar.activation(
                    out=dst, in_=src,
                    func=mybir.ActivationFunctionType.Prelu,
                    bias=bias_mid, scale=asc, alpha=alpha,
                )


def _kernel_body(tc, out_d, x_d, w3_d, wpw_d, vec_d, bl, loop_n=None):
    nc = tc.nc
    ctx = ExitStack()
    with ctx:
        const = ctx.enter_context(tc.tile_pool(name="const", bufs=1))
        s1pool = ctx.enter_context(tc.tile_pool(name="s1pool", bufs=2))
        p1pool = ctx.enter_context(tc.tile_pool(name="p1pool", bufs=2))
        s2pool = ctx.enter_context(tc.tile_pool(name="s2pool", bufs=2))
        p2pool = ctx.enter_context(tc.tile_pool(name="p2pool", bufs=P2_BUFS))
        # [128, 1024] psum tiles = 2 banks each; 4 bufs = all 8 banks
        ps = ctx.enter_context(tc.tile_pool(name="ps", bufs=4, space="PSUM"))

        # ---- constants -------------------------------------------------
        v = {}
        for n in WVEC_NAMES:
            v[n] = const.tile([128, 1], F32, name=f"v_{n}")
            nc.sync.dma_start(out=v[n], in_=_bcast_ap(vec_d[n]))
        b31 = const.tile([128, 1], F32, name="b31")  # b13 + b21
        nc.vector.tensor_tensor(
            out=b31, in0=v["b13"], in1=v["b21"], op=mybir.AluOpType.add
        )
        b32 = const.tile([128, 1], F32, name="b32")  # b13 + b22
        nc.vector.tensor_tensor(
            out=b32, in0=v["b13"], in1=v["b22"], op=mybir.AluOpType.add
        )
        # DVE signs compare against the negated bias: s = (u >= -b) - 0.5
        nb31 = const.tile([128, 1], F32, name="nb31")
        nc.vector.tensor_scalar_mul(nb31, b31, -1.0)
        nb11 = const.tile([128, 1], F32, name="nb11")
        nc.vector.tensor_scalar_mul(nb11, v["b11"], -1.0)

        w1, as1, id1 = _prep_conv_consts(nc, const, w3_d, "c1",
                                         samp=0.5 if S1_DVE else 1.0)
        w2, as2, id2 = _prep_conv_consts(nc, const, wpw_d, "c2",
                                         samp=0.5 if S2_DVE else 1.0)

        loop_cm = tc.For_i(0, loop_n, 1) if loop_n else nullcontext()
        with loop_cm:
            _main_strips(tc, nc, out_d, x_d, bl, v, b31, b32, nb11, nb31,
                         w1, as1, id1, w2, as2, id2,
                         s1pool, p1pool, s2pool, p2pool, ps)


def _main_strips(tc, nc, out_d, x_d, bl, v, b31, b32, nb11, nb31,
                 w1, as1, id1, w2, as2, id2,
                 s1pool, p1pool, s2pool, p2pool, ps):
    X_ROWS = R + 4     # x / s1 strip rows   [h0-2, h0+R+2)
    P_ROWS = R + 2     # p1 / s2 strip rows  [h0-1, h0+R+1)
    X_LEN = X_ROWS * WP
    P_LEN = P_ROWS * WP

    # groups of 4 images (quad) or 2 (pair, AB tiles only — sim harness)
    groups = []
    i = 0
    while i < bl:
        g = min(4, bl - i)
        assert g in (2, 4)
        groups.append(list(range(i, i + g)))
        i += g

    for imgs in groups:
        nquad = len(imgs) == 4
        ngr = len(imgs) // 2
        prev_p1_ts = None

        def _strip_rows(si):
            h0i = si * R
            c1s = max(h0i - 1, 0) if si == 0 else h0i + 1
            c1h = min(h0i + R + 1, H)
            lo_l = max(c1s - 1, 0) - (h0i - 2)
            hi_l = min(c1h + 1, H) - (h0i - 2)
            return lo_l, hi_l

        # x staging: UNPADDED bf16 with one lead element -- elem(r, c) =
        # 1 + r*W + c.  The cast-DMA lands as one contiguous run per
        # partition; the sign and the residual matmul read it through
        # 114-wide windows at 112-row-stride (overlapping APs): the wrapped
        # edge elements produce garbage that only ever reaches discarded
        # PSUM pad columns / re-zeroed s1 pad columns.
        XU_LEN = X_ROWS * W + 2

        def load_xb(si):
            # SWDGE cast DMA: fp32 HBM -> bf16, one 128-partition transfer
            # per image pair; emitted one strip AHEAD to hide under compute
            h0i = si * R
            lo_l, hi_l = _strip_rows(si)
            xloi = lo_l + h0i - 2
            xhii = hi_l + h0i - 2
            tiles = []
            for gi in range(ngr):
                xb_t = s1pool.tile([128, XU_LEN], BF16, tag=f"xb{gi}",
                                   name="xb")
                if not SKIP_LOAD:
                    src = x_d[
                        imgs[2 * gi] : imgs[2 * gi] + 2, :, xloi:xhii, :
                    ].rearrange("i c r w -> (i c) (r w)")
                    nc.gpsimd.dma_start(
                        out=xb_t[:, 1 + lo_l * W : 1 + hi_l * W], in_=src
                    )
                else:
                    nc.gpsimd.memset(xb_t[:, 1 + lo_l * W : 1 + hi_l * W],
                                     0.5)
                # the two border elements the overlapping windows touch
                # beyond the loaded rows
                nc.gpsimd.memset(xb_t[:, lo_l * W : lo_l * W + 1], 0.0)
                nc.gpsimd.memset(
                    xb_t[:, 1 + hi_l * W : 2 + hi_l * W], 0.0
                )
                tiles.append(xb_t)
            return tiles

        def _xwin(xb_t, rsl, r0_l, nrows):
            # overlapping window AP: row k of the result reads unpadded
            # elems [(r0_l+k)*W .. +114) = [wrap | x[r0_l+k,:] | wrap]
            v = xb_t[rsl, r0_l * W : r0_l * W + 1]
            return bass.AP(
                tensor=v.tensor, offset=v.offset,
                ap=[list(v.ap[0]), [W, nrows], [1, WP]],
            )

        def prep_s1(si, xb_ts_i):
            # s1 = sign(x + b11) as +-0.5 over full padded rows via the
            # overlapping window (contiguous fast-mode DVE op; pad cols
            # catch the wrapped garbage and are re-zeroed below)
            lo_l, hi_l = _strip_rows(si)
            tiles = []
            for gi, xb_t in enumerate(xb_ts_i):
                s1_t = s1pool.tile([128, X_LEN + 4], BF16, tag=f"s1{gi}",
                                   name="s1")
                s1_r = s1_t[:, 2 : 2 + X_LEN].rearrange(
                    "p (r c) -> p r c", c=WP
                )
                full = slice(0, 128)
                win = _xwin(xb_t, full, lo_l, hi_l - lo_l)
                dst = s1_t[:, 2 + lo_l * WP : 2 + hi_l * WP]
                if S1_DVE:
                    nc.vector.tensor_scalar(
                        out=dst, in0=win,
                        scalar1=nb11, scalar2=0.5,
                        op0=mybir.AluOpType.is_ge,
                        op1=mybir.AluOpType.subtract,
                    )
                else:
                    nc.scalar.activation(
                        out=dst, in_=win,
                        func=mybir.ActivationFunctionType.Sign,
                        bias=v["b11"],
                    )
                # full-row sign poisons pad cols: re-zero every strip
                nc.gpsimd.memset(s1_r[:, :, 0:1], 0.0)
                nc.gpsimd.memset(s1_r[:, :, WP - 1 : WP], 0.0)
                nc.gpsimd.memset(s1_t[:, 0:2], 0.0)
                nc.gpsimd.memset(s1_t[:, 2 + X_LEN :], 0.0)
                if si == 0 and lo_l > 0:  # top image edge
                    nc.gpsimd.memset(s1_t[:, 2 : 2 + lo_l * WP], 0.0)
                if hi_l < X_ROWS:  # bottom image edge
                    nc.gpsimd.memset(
                        s1_t[:, 2 + hi_l * WP : 2 + X_LEN], 0.0
                    )
                tiles.append(s1_t)
            return tiles

        xb_ts = load_xb(0)
        s1_ts = prep_s1(0, xb_ts)
        for s in range(NSTRIPS):
            h0 = s * R
            c1lo, c1hi = max(h0 - 1, 0), min(h0 + R + 1, H)
            # rows computed by conv1 this strip; rows [h0-1, h0+1) are
            # carried over from the previous strip's p1 (no halo recompute)
            c1start = c1lo if s == 0 else h0 + 1

            def xloc(g):   # global row -> local row in x/s1 strip
                return g - (h0 - 2)

            def ploc(g):   # global row -> local row in p1/s2 strip
                return g - (h0 - 1)

            xb_next = load_xb(s + 1) if s + 1 < NSTRIPS else None

            # -- conv1 + fused residual/scale/bias/prelu -----------------
            # p1_AB = [p1_A(lo); p1_B(hi)]; p1_CD = [p1_D(lo); p1_C(hi)]
            # bf16: feeds Sign + the conv2 residual matmul (1 cyc/row)
            p1_ts = [
                p1pool.tile([128, P_LEN + 4], BF16, tag=f"p1{gi}", name="p1")
                for gi in range(ngr)
            ]

            # carry rows [h0-1, h0+1) of p1 from the previous strip
            if s > 0:
                for gi in range(ngr):
                    nc.vector.tensor_copy(
                        out=p1_ts[gi][:, 2 : 2 + 2 * WP],
                        in_=prev_p1_ts[gi][:, 2 + R * WP : 2 + (R + 2) * WP],
                    )

            for r0, nr in _row_chunks(c1start, c1hi, step=8):
                soff_of = (
                    lambda rsl, t, _r0=r0: 2
                    + (xloc(_r0) + t // 3 - 1) * WP + (t % 3 - 1)
                )
                r0_l = xloc(r0)

                def res1_of(gi):  # unpadded x through the overlap window
                    return lambda rsl, dr, snr, _g=gi: _xwin(
                        xb_ts[_g], rsl, r0_l + dr, snr
                    )

                o1 = slice(2 + ploc(r0) * WP, 2 + ploc(r0) * WP + nr * WP)
                quads = [
                    # imgA: aligned lo->lo (pair-tile 0 lo)
                    (LO, LO, s1_ts[0], res1_of(0)),
                    # imgB: aligned hi->hi (pair-tile 0 hi)
                    (HI, HI, s1_ts[0], res1_of(0)),
                ]
                pair_outs = [p1_ts[0][:, o1]]
                if nquad:
                    quads += [
                        # imgC: crossed lo->hi (home flips to hi for conv2)
                        (LO, HI, s1_ts[1], res1_of(1)),
                        # imgD: crossed hi->lo
                        (HI, LO, s1_ts[1], res1_of(1)),
                    ]
                    pair_outs.append(p1_ts[1][:, o1])
                _conv_quad(nc, ps, w1, id1, quads, pair_outs, soff_of, nr,
                           as1, v["b12"], v["a1"])

            # -- s2 = sign(p1 + b13 + b21), zero padding -----------------
            # signed in chunk-sized pieces so conv2 chunks can start as
            # soon as their input rows are evicted (no whole-strip barrier)
            s2_pieces = ([(ploc(h0 - 1), 2)] if s > 0 else []) + [
                (ploc(r0), nr) for r0, nr in _row_chunks(c1start, c1hi, step=8)
            ]
            s2_ts = []
            for gi, p1_t in enumerate(p1_ts):
                s2_t = s2pool.tile([128, P_LEN + 4], BF16, tag=f"s2{gi}",
                                   name="s2")
                s2_r = s2_t[:, 2 : 2 + P_LEN].rearrange(
                    "p (r c) -> p r c", c=WP
                )
                for pr, pn in s2_pieces:
                    if S2_DVE:
                        nc.vector.tensor_scalar(
                            out=s2_t[:, 2 + pr * WP : 2 + (pr + pn) * WP],
                            in0=p1_t[:, 2 + pr * WP : 2 + (pr + pn) * WP],
                            scalar1=nb31, scalar2=0.5,
                            op0=mybir.AluOpType.is_ge,
                            op1=mybir.AluOpType.subtract,
                        )
                    else:
                        nc.scalar.activation(
                            out=s2_t[:, 2 + pr * WP : 2 + (pr + pn) * WP],
                            in_=p1_t[:, 2 + pr * WP : 2 + (pr + pn) * WP],
                            func=mybir.ActivationFunctionType.Sign,
                            bias=b31,
                        )
                # the sign writes full rows incl. pad cols: re-zero each strip
                nc.gpsimd.memset(s2_r[:, :, 0:1], 0.0)
                nc.gpsimd.memset(s2_r[:, :, WP - 1 : WP], 0.0)
                nc.gpsimd.memset(s2_t[:, 0:2], 0.0)
                nc.gpsimd.memset(s2_t[:, 2 + P_LEN :], 0.0)
                if ploc(c1lo) > 0:
                    nc.gpsimd.memset(s2_t[:, 2 : 2 + ploc(c1lo) * WP], 0.0)
                if ploc(c1hi) < P_ROWS:
                    nc.gpsimd.memset(
                        s2_t[:, 2 + ploc(c1hi) * WP : 2 + P_LEN], 0.0
                    )
                s2_ts.append(s2_t)

            # next strip's sign, pipelined mid-strip: the DVE runs it after
            # this strip's s2 pieces, well before conv1(s+1) needs it
            s1_next = prep_s1(s + 1, xb_next) if s + 1 < NSTRIPS else None

            # -- conv2 + fused chain -------------------------------------
            # p2_AB = [A(lo); B(hi)]; p2_CD = [C(lo); D(hi)] (crossed back)
            # unpadded bf16: evictions strip pad cols; output cast to f32
            # on the host
            p2_ts = [
                p2pool.tile([128, R * W], BF16, tag=f"p2{gi}", name="p2")
                for gi in range(ngr)
            ]
            p2_rs = [t.rearrange("p (r c) -> p r c", c=W) for t in p2_ts]
            for r0, nr in _row_chunks(h0, h0 + R, step=8):
                soff_of = (
                    lambda rsl, t, _r0=r0: 2
                    + (ploc(_r0) + t // 3 - 1) * WP + (t % 3 - 1)
                )
                roff = 2 + ploc(r0) * WP

                def res2_of(gi):  # padded p1, flat slice
                    return lambda rsl, dr, snr, _g=gi: p1_ts[_g][
                        rsl, roff + dr * WP : roff + (dr + snr) * WP
                    ]

                r2 = slice(r0 - h0, r0 - h0 + nr)
                quads = [
                    (LO, LO, s2_ts[0], res2_of(0)),
                    (HI, HI, s2_ts[0], res2_of(0)),
                ]
                pair_outs = [p2_rs[0][:, r2, :]]
                if nquad:
                    quads += [
                        # imgC now lives on hi; crossed hi->lo back home
                        (HI, LO, s2_ts[1], res2_of(1)),
                        # imgD on lo; crossed lo->hi
                        (LO, HI, s2_ts[1], res2_of(1)),
                    ]
                    pair_outs.append(p2_rs[1][:, r2, :])
                _conv_quad(nc, ps, w2, id2, quads, pair_outs, soff_of, nr,
                           as2, b32, v["a2"], strip_pads=True)

                # out2 = p2 + b23, per chunk (spreads elementwise work)
                b23_eng = nc.gpsimd if B23_GPSIMD else nc.vector
                for gi in range(ngr):
                    p2_r = p2_rs[gi]
                    b23_eng.tensor_scalar_add(
                        p2_r[:, r2, :], p2_r[:, r2, :], v["b23"]
                    )

            # -- store: one merged 128-partition HWDGE DMA per pair ------
            for gi in range(ngr):
                if not SKIP_STORE:
                    dst = out_d[
                        imgs[2 * gi] : imgs[2 * gi] + 2, :, h0 : h0 + R, :
                    ].rearrange("i c r w -> (i c) r w")
                    nc.sync.dma_start(out=dst, in_=p2_rs[gi])
            prev_p1_ts = p1_ts
            xb_ts = xb_next
            s1_ts = s1_next


_NC_CACHE = {}


def _get_program(bl=BL):
    if bl not in _NC_CACHE:
        _NC_CACHE[bl] = build_program(bl)
    return _NC_CACHE[bl]


def make_in_maps(inputs):
    x = np.ascontiguousarray(np.asarray(inputs["x"], dtype=np.float32))
    shared = {
        "w3": np.ascontiguousarray(np.asarray(inputs["w3"], np.float32)),
        "w_pw": np.ascontiguousarray(np.asarray(inputs["w_pw"], np.float32)),
    }
    for n in WVEC_NAMES:
        shared[n] = np.ascontiguousarray(np.asarray(inputs[n], np.float32))
    return [{"x": x[i * BL : (i + 1) * BL], **shared} for i in range(NCORES)]


def run(inputs, trace=False, **kwargs):
    nc = _get_program(BL)
    res = run_bass_kernel_spmd(
        nc, make_in_maps(inputs), core_ids=list(range(NCORES)), trace=trace,
        **kwargs,
    )
    out = np.concatenate(
        [np.asarray(r["out"], dtype=np.float32) for r in res.results], axis=0
    )
    return out, res


def kernel(**inputs):
    return run(inputs)[0]


def bench(inputs, iters=20, nc=None):
    """Steady-state wall-clock benchmark: sharded jit without donation,
    device-resident inputs, async dispatch of `iters` executions."""
    import time
    import jax
    from jax.sharding import Mesh, PartitionSpec, NamedSharding
    from jax.experimental.shard_map import shard_map
    from concourse import bass2jax as b2j

    b2j.install_neuronx_cc_hook()
    if nc is None:
        nc = _get_program(BL)
    in_maps = make_in_maps(inputs)

    in_names, out_names, out_avals = [], [], []
    for alloc in nc.m.functions[0].allocations:
        if not isinstance(mybir.MemoryLocationSet, type) or not isinstance(
            alloc, mybir.MemoryLocationSet
        ):
            continue
        name = alloc.memorylocations[0].name
        if alloc.kind == "ExternalInput":
            if nc.partition_id_tensor and name == nc.partition_id_tensor.name:
                continue
            in_names.append(name)
        elif alloc.kind == "ExternalOutput":
            out_names.append(name)
            out_avals.append(
                jax.core.ShapedArray(
                    tuple(alloc.tensor_shape), mybir.dt.np(alloc.dtype)
                )
            )
    n_params = len(in_names)
    all_names = in_names + out_names
    if nc.partition_id_tensor:
        all_names = all_names + [nc.partition_id_tensor.name]

    def _body(*args):
        operands = list(args)
        if nc.partition_id_tensor:
            operands.append(b2j.partition_id_tensor())
        outs = b2j._bass_exec_p.bind(
            *operands,
            out_avals=tuple(out_avals),
            in_names=tuple(all_names),
            out_names=tuple(out_names),
            lowering_input_output_aliases=(),
            sim_require_finite=True,
            sim_require_nnan=True,
            nc=nc,
        )
        return tuple(outs)

    devices = jax.devices()[:NCORES]
    mesh = Mesh(np.asarray(devices), ("core",))
    nin = n_params + len(out_names)
    f = jax.jit(
        shard_map(
            _body,
            mesh=mesh,
            in_specs=(PartitionSpec("core"),) * nin,
            out_specs=(PartitionSpec("core"),) * len(out_names),
            check_rep=False,
        ),
        keep_unused=True,
    )
    sh = NamedSharding(mesh, PartitionSpec("core"))
    concat_in = [
        jax.device_put(np.concatenate([m[n] for m in in_maps], axis=0), sh)
        for n in in_names
    ]
    zeros = [
        jax.device_put(
            np.zeros((NCORES * a.shape[0], *a.shape[1:]), a.dtype), sh
        )
        for a in out_avals
    ]

    r = f(*concat_in, *zeros)  # warm-up / compile
    jax.block_until_ready(r)

    ts = []
    for _ in range(max(iters, 8)):
        t0 = time.perf_counter()
        r = f(*concat_in, *zeros)
        jax.block_until_ready(r)
        ts.append(time.perf_counter() - t0)
    return {"single_s": min(ts), "all": ts}


def bench_device(inputs, loops=(64, 1024), calls=10):
    """Per-iteration device time via on-device For_i repetition.  The two
    loop-count programs are dispatched in interleaved alternation so slow
    drift in dispatch overhead cancels out of the slope."""
    import time
    import jax
    from jax.sharding import Mesh, PartitionSpec, NamedSharding

    fns = {}
    for L in loops:
        nc = build_program(BL, loop_n=L)
        fns[L] = _bench_fn(inputs, nc)
    ts = {L: [] for L in loops}
    for L in loops:  # warm-up / compile
        jax.block_until_ready(fns[L]())
    for _ in range(calls):
        for L in loops:
            t0 = time.perf_counter()
            jax.block_until_ready(fns[L]())
            ts[L].append(time.perf_counter() - t0)
    res = {L: min(v) for L, v in ts.items()}
    for L in loops:
        print(f"  loop_n={L}: best single call {res[L] * 1e3:.2f} ms")
    l0, l1 = loops
    per_iter = (res[l1] - res[l0]) / (l1 - l0)
    return {"per_iter_s": per_iter, "times": res}


def _bench_fn(inputs, nc):
    """Build a zero-copy dispatch closure for `nc` (device-resident args)."""
    import jax
    from jax.sharding import Mesh, PartitionSpec, NamedSharding
    from jax.experimental.shard_map import shard_map
    from concourse import bass2jax as b2j

    b2j.install_neuronx_cc_hook()
    in_maps = make_in_maps(inputs)
    in_names, out_names, out_avals = [], [], []
    for alloc in nc.m.functions[0].allocations:
        if not isinstance(alloc, mybir.MemoryLocationSet):
            continue
        name = alloc.memorylocations[0].name
        if alloc.kind == "ExternalInput":
            if nc.partition_id_tensor and name == nc.partition_id_tensor.name:
                continue
            in_names.append(name)
        elif alloc.kind == "ExternalOutput":
            out_names.append(name)
            out_avals.append(
                jax.core.ShapedArray(
                    tuple(alloc.tensor_shape), mybir.dt.np(alloc.dtype)
                )
            )
    all_names = in_names + out_names
    if nc.partition_id_tensor:
        all_names = all_names + [nc.partition_id_tensor.name]

    def _body(*args):
        operands = list(args)
        if nc.partition_id_tensor:
            operands.append(b2j.partition_id_tensor())
        return tuple(
            b2j._bass_exec_p.bind(
                *operands,
                out_avals=tuple(out_avals),
                in_names=tuple(all_names),
                out_names=tuple(out_names),
                lowering_input_output_aliases=(),
                sim_require_finite=True,
                sim_require_nnan=True,
                nc=nc,
            )
        )

    devices = jax.devices()[:NCORES]
    mesh = Mesh(np.asarray(devices), ("core",))
    nin = len(in_names) + len(out_names)
    f = jax.jit(
        shard_map(
            _body, mesh=mesh,
            in_specs=(PartitionSpec("core"),) * nin,
            out_specs=(PartitionSpec("core"),) * len(out_names),
            check_rep=False,
        ),
        keep_unused=True,
    )
    sh = NamedSharding(mesh, PartitionSpec("core"))
    concat_in = [
        jax.device_put(np.concatenate([m[n] for m in in_maps], axis=0), sh)
        for n in in_names
    ]
    zeros = [
        jax.device_put(
            np.zeros((NCORES * a.shape[0], *a.shape[1:]), a.dtype), sh
        )
        for a in out_avals
    ]
    return lambda: f(*concat_in, *zeros)


if __name__ == "__main__":
    rng = np.random.default_rng(0)
    ins = {"x": rng.standard_normal((B, C, H, W)).astype(np.float32)}
    for n in ["w3", "w_pw"]:
        ins[n] = ((rng.random((C, C, 3, 3)) - 0.5) * 0.002).astype(np.float32)
    for n in WVEC_NAMES:
        ins[n] = (rng.standard_normal(C) * 0.01).astype(np.float32)
    out = kernel(**ins)
    print(out.shape, out.dtype)



# revision 34
# speedup vs baseline: 2.7817x; 2.7817x over previous
"""Trainium2 Bass kernel for a ReActNet-style binary BasicBlock.

Reference math per block (twice, with different weights):
    s   = sign(x + b_in)                      # +-1
    c   = conv3x3(s, mean|w| * sign(w))       # binarized conv, pad=1
    y   = x + ALPHA * c                       # residual
    y   = prelu(y + b_mid, a) + b_out

Key facts exploited:
  * matmul inputs are exactly +-1 -> bf16 matmuls are EXACT (integer sums
    accumulated in fp32 PSUM).
  * per-output-channel weight scale factors out:  conv(s, scale*sign(w)) =
    scale .* conv(s, sign(w)).
  * residual rides through PSUM via a bf16 matmul with diag(1/(ALPHA*scale))
    (bf16 streams 1 cycle/row on the PE; fp32 costs 4): x and p1 are held /
    evicted as bf16 (~0.4% rounding, well inside the 2e-2 gate).
        T = binconv(s) + x / as           (as = ALPHA*scale, per channel)
    then prelu(x + as*binconv + b, a) = Prelu-activation(T) with
    per-partition scale=as, bias=b, alpha=a  -- a single ScalarE op reading
    PSUM directly.  (prelu positive homogeneity: as > 0.)

Layout (q4i scheme): NCHW, channels (64) on partitions; FOUR images in
flight on the four 64x64 PE tiles, PAIRED into two PSUM banks (diagonal
row-split pairs write complementary partition halves of one bank; the
per-partition has_written zero-regions keep the accumulation groups
independent):
    imgA: tile(0,0)   rows 0-63  -> bankX 0-63    (aligned)
    imgB: tile(64,64) rows 64-127-> bankX 64-127  (aligned)
    imgC: tile(0,64) / tile(64,0) alternating per conv -> bankY (crossed)
    imgD: tile(64,0) / tile(0,64) alternating per conv -> bankY (crossed)
The crossed tiles flip C/D's partition home after each conv; all engine ops
stay partition-aligned, DMA handles the final placement for free.
Bank pairing lets every PSUM eviction run as ONE 128-partition ScalarE
Prelu op per pair (vs two 64-partition ops): half the ACT instructions at
full lane utilization.

Engine balance: signs run on the DVE as single tensor_scalar ops
producing +-0.5 ((x >= -b) - 0.5); the missing x2 is folded into the
per-channel eviction scale (asc = 2*ALPHA*mean|w|), keeping matmuls
exact.  Store DMAs issue from the GpSimd queue, loads from Sync, so the
ScalarE queue does nothing but the two fused Prelu evictions per chunk.

Spatial strips of R=16 output rows.  x is DMA'd into contiguous unpadded
staging (8.9KB descriptor runs, vs 448B for a padded layout) and restrided
to the padded-114 conv layout by the Sign / bf16-cast ops whose in/out APs
differ anyway; p2 is evicted unpadded for the same reason on the store
side.  conv1 rows [h0-1,h0+1) are carried over from the previous strip's
p1 (no halo recompute); s2 is signed in chunk-sized pieces so conv2 can
start before the whole strip is evicted.

Sharding: pure data parallel, batch 32 -> 4 images x 8 cores, weights
replicated, no collectives.
"""

import numpy as np
from contextlib import ExitStack, nullcontext

import concourse.bass as bass
import concourse.tile as tile
from concourse import mybir
from concourse import bacc
from concourse.bass_utils import run_bass_kernel_spmd
from concourse.masks import make_identity

B, C, H, W = 32, 64, 112, 112
ALPHA = 0.25
NCORES = 8
BL = B // NCORES          # images per core
WP = W + 2                # padded width
R = 16                    # output rows per strip
NSTRIPS = H // R

F32 = mybir.dt.float32
BF16 = mybir.dt.bfloat16

WVEC_NAMES = ["b11", "b12", "b13", "b21", "b22", "b23", "a1", "a2"]

SKIP_STORE = False   # timing experiment: drop output DMA
SKIP_LOAD = False    # timing experiment: drop input DMA (garbage data)
RESID_LAST = True    # residual matmul in slot 9 (vs slot 0)
P2_BUFS = 2          # p2 double/triple buffering
S1_DVE = True        # s1 sign on DVE (else ScalarE Sign, +-1 w/ asc1 x1)
S2_DVE = True        # s2 sign on DVE (else ScalarE Sign)
B23_GPSIMD = False   # final +b23 on GpSimd (Pool tensor_scalar: ~15x
                     # slower than DVE -- measured 13us/op; keep False)


def _bcast_ap(dram_ap, reps=2):
    """Source AP replicating a DRAM tensor across partition groups."""
    return bass.AP(
        tensor=dram_ap.tensor,
        offset=dram_ap.offset,
        ap=[[0, reps]] + [list(d) for d in dram_ap.ap],
    )


def _row_chunks(lo, hi, step=4):
    r = lo
    while r < hi:
        yield r, min(step, hi - r)
        r += step


def build_program(bl=BL, loop_n=None):
    """Build the Bass program for one core processing `bl` images.

    loop_n: if set, repeat the whole main loop on-device that many times
    (timing harness only -- results identical, just recomputed).
    """
    nc = bacc.Bacc("TRN2", target_bir_lowering=False, debug=False)

    x_d = nc.dram_tensor("x", [bl, C, H, W], F32, kind="ExternalInput").ap()
    w3_d = nc.dram_tensor("w3", [C, C, 3, 3], F32, kind="ExternalInput").ap()
    wpw_d = nc.dram_tensor("w_pw", [C, C, 3, 3], F32, kind="ExternalInput").ap()
    vec_d = {
        n: nc.dram_tensor(n, [C], F32, kind="ExternalInput").ap()
        for n in WVEC_NAMES
    }
    # bf16 output (host casts back to f32): halves the store traffic and
    # stays well inside the 2e-2 gate (adds <= 2^-9 relative rounding)
    out_d = nc.dram_tensor("out", [bl, C, H, W], BF16, kind="ExternalOutput").ap()

    with tile.TileContext(nc) as tc:
        _kernel_body(tc, out_d, x_d, w3_d, wpw_d, vec_d, bl, loop_n=loop_n)

    nc.compile()
    return nc


def _prep_conv_consts(nc, const, wdram, name, samp=1.0):
    """Per-conv constants: binarized-transposed weights, the eviction scale
    asc = ALPHA*mean|w|/samp (samp = sign amplitude: 0.5 when the DVE sign
    produces +-0.5), diag(1/asc) for the residual matmul.  Everything
    replicated on both partition halves."""
    # natural layout [co, ci*3*3] duplicated -> per-channel scale
    wn = const.tile([128, C * 9], F32, name=f"wn_{name}")
    nc.sync.dma_start(out=wn, in_=_bcast_ap(wdram.rearrange("a b c d -> a (b c d)")))
    wabs = const.tile([128, C * 9], F32, name=f"wabs_{name}")
    asum = const.tile([128, 1], F32, name=f"asum_{name}")
    nc.scalar.activation(
        out=wabs, in_=wn, func=mybir.ActivationFunctionType.Abs, accum_out=asum
    )
    asc = const.tile([128, 1], F32, name=f"asc_{name}")
    nc.vector.tensor_scalar_mul(asc, asum, ALPHA / (C * 9) / samp)
    inv_asc = const.tile([128, 1], F32, name=f"iasc_{name}")
    nc.vector.reciprocal(inv_asc, asc)

    # gathered+transposed weights [ci(+dup), co, tap], then binarize to bf16
    wg = const.tile([128, C, 9], F32, name=f"wg_{name}")
    src = bass.AP(
        tensor=wdram.tensor,
        offset=wdram.offset,
        ap=[[9, C], [C * 9, C], [1, 9]],
    )
    for rep in range(2):
        nc.sync.dma_start(out=wg[64 * rep : 64 * rep + 64, :, :], in_=src)
    wsign = const.tile([128, C, 9], BF16, name=f"ws_{name}")
    nc.scalar.activation(out=wsign, in_=wg, func=mybir.ActivationFunctionType.Sign)

    # residual injector: diag(1/as) bf16 (bf16 matmul = 1 cyc/row vs 4 for
    # fp32), per partition half
    ident = const.tile([128, C], BF16, name=f"id_{name}")
    make_identity(nc, ident[0:64, :])
    make_identity(nc, ident[64:128, :])
    nc.vector.tensor_scalar_mul(ident, ident, inv_asc)
    return wsign, asc, ident


LO = slice(0, 64)
HI = slice(64, 128)


def _conv_quad(nc, ps, w, ident, quads, pair_outs, soff_of, nr, asc, bias_mid,
               alpha, strip_pads=False):
    """One double-chunk (up to 8 output rows) of conv for four images on the
    four 64x64 PE tiles.

    quads: list of 4 tuples (rsl, osl, s_t, res_of):
      rsl: SBUF row half this image's data lives on (slice)
      osl: PSUM partition half this image's results land on (slice)
      s_t: sign tile; res_of(rsl, dr, snr): residual rhs AP for a sub-chunk
    Images 0,1 share a PSUM tile ([128, 1024] = 2 banks) and images 2,3 a
    second one (complementary partition halves; per-partition has_written
    keeps the accumulation groups independent, and the diagonal tile pairs
    stream concurrently).  Rows split into two <=4-row sub-chunks, one per
    bank (cols 0:456 and 512:968), so a full 8-row double-chunk evicts as
    ONE 128-partition ACT per pair reading both banks.
    Slots: the 9 taps then the residual-injector matmul.
    Eviction: out = Prelu(asc*psum + bias) per pair.
    """
    npair = (len(quads) + 1) // 2
    pts = [ps.tile([128, 1024], F32, tag="ps", name="pt") for _ in range(npair)]
    subs = [(0, min(4, nr))] + ([(4, nr - 4)] if nr > 4 else [])
    rslot = 9 if RESID_LAST else 0
    for s in range(10):
        for qi, (rsl, osl, s_t, res_of) in enumerate(quads):
            pt = pts[qi // 2]
            for dr, snr in subs:
                nn = snr * WP
                pc = 0 if dr == 0 else 512
                if s == rslot:
                    # residual slot: taps elsewhere need only s_t
                    nc.tensor.matmul(
                        pt[osl, pc : pc + nn], ident[rsl, :],
                        res_of(rsl, dr, snr),
                        start=(s == 0), stop=(s == 9), skip_group_check=True,
                    )
                else:
                    t = s - 1 if s > rslot else s
                    so = soff_of(rsl, t) + dr * WP
                    nc.tensor.matmul(
                        pt[osl, pc : pc + nn], w[rsl, :, t],
                        s_t[rsl, so : so + nn],
                        start=(s == 0), stop=(s == 9), skip_group_check=True,
                    )
    for pt, out_ap in zip(pts, pair_outs):
        # out_ap covers the pair's full nr rows: flat [128, nr*WP] (conv1,
        # padded) or [128, nr, W] (conv2, pads stripped)
        if nr == 8:  # one ACT reading both banks
            if strip_pads:
                src = bass.AP(
                    tensor=pt.tensor, offset=pt.offset + 1,
                    ap=[list(pt.ap[0]), [512, 2], [WP, 4], [1, W]],
                )
            else:
                src = bass.AP(
                    tensor=pt.tensor, offset=pt.offset,
                    ap=[list(pt.ap[0]), [512, 2], [1, 456]],
                )
            nc.scalar.activation(
                out=out_ap, in_=src,
                func=mybir.ActivationFunctionType.Prelu,
                bias=bias_mid, scale=asc, alpha=alpha,
            )
        else:
            for dr, snr in subs:
                pc = 0 if dr == 0 else 512
                src = pt[:, pc : pc + snr * WP]
                if strip_pads:
                    src = src.rearrange("p (r c) -> p r c", c=WP)[:, :, 1 : 1 + W]
                    dst = out_ap[:, dr : dr + snr, :]
                else:
                    dst = out_ap[:, dr * WP : (dr + snr) * WP]
                nc.scalar.activation(
                    out=dst, in_=src,
                    func=mybir.ActivationFunctionType.Prelu,
                    bias=bias_mid, scale=asc, alpha=alpha,
                )


def _kernel_body(tc, out_d, x_d, w3_d, wpw_d, vec_d, bl, loop_n=None):
    nc = tc.nc
    ctx = ExitStack()
    with ctx:
        const = ctx.enter_context(tc.tile_pool(name="const", bufs=1))
        s1pool = ctx.enter_context(tc.tile_pool(name="s1pool", bufs=2))
        p1pool = ctx.enter_context(tc.tile_pool(name="p1pool", bufs=2))
        s2pool = ctx.enter_context(tc.tile_pool(name="s2pool", bufs=2))
        p2pool = ctx.enter_context(tc.tile_pool(name="p2pool", bufs=P2_BUFS))
        # [128, 1024] psum tiles = 2 banks each; 4 bufs = all 8 banks
        ps = ctx.enter_context(tc.tile_pool(name="ps", bufs=4, space="PSUM"))

        # ---- constants -------------------------------------------------
        v = {}
        for n in WVEC_NAMES:
            v[n] = const.tile([128, 1], F32, name=f"v_{n}")
            nc.sync.dma_start(out=v[n], in_=_bcast_ap(vec_d[n]))
        b31 = const.tile([128, 1], F32, name="b31")  # b13 + b21
        nc.vector.tensor_tensor(
            out=b31, in0=v["b13"], in1=v["b21"], op=mybir.AluOpType.add
        )
        b32 = const.tile([128, 1], F32, name="b32")  # b13 + b22
        nc.vector.tensor_tensor(
            out=b32, in0=v["b13"], in1=v["b22"], op=mybir.AluOpType.add
        )
        # DVE signs compare against the negated bias: s = (u >= -b) - 0.5
        nb31 = const.tile([128, 1], F32, name="nb31")
        nc.vector.tensor_scalar_mul(nb31, b31, -1.0)
        nb11 = const.tile([128, 1], F32, name="nb11")
        nc.vector.tensor_scalar_mul(nb11, v["b11"], -1.0)

        w1, as1, id1 = _prep_conv_consts(nc, const, w3_d, "c1",
                                         samp=0.5 if S1_DVE else 1.0)
        w2, as2, id2 = _prep_conv_consts(nc, const, wpw_d, "c2",
                                         samp=0.5 if S2_DVE else 1.0)

        loop_cm = tc.For_i(0, loop_n, 1) if loop_n else nullcontext()
        with loop_cm:
            _main_strips(tc, nc, out_d, x_d, bl, v, b31, b32, nb11, nb31,
                         w1, as1, id1, w2, as2, id2,
                         s1pool, p1pool, s2pool, p2pool, ps)


def _main_strips(tc, nc, out_d, x_d, bl, v, b31, b32, nb11, nb31,
                 w1, as1, id1, w2, as2, id2,
                 s1pool, p1pool, s2pool, p2pool, ps):
    X_ROWS = R + 4     # x / s1 strip rows   [h0-2, h0+R+2)
    P_ROWS = R + 2     # p1 / s2 strip rows  [h0-1, h0+R+1)
    X_LEN = X_ROWS * WP
    P_LEN = P_ROWS * WP

    # groups of 4 images (quad) or 2 (pair, AB tiles only — sim harness)
    groups = []
    i = 0
    while i < bl:
        g = min(4, bl - i)
        assert g in (2, 4)
        groups.append(list(range(i, i + g)))
        i += g

    for imgs in groups:
        nquad = len(imgs) == 4
        ngr = len(imgs) // 2
        prev_p1_ts = None

        def _strip_rows(si):
            h0i = si * R
            c1s = max(h0i - 1, 0) if si == 0 else h0i + 1
            c1h = min(h0i + R + 1, H)
            lo_l = max(c1s - 1, 0) - (h0i - 2)
            hi_l = min(c1h + 1, H) - (h0i - 2)
            return lo_l, hi_l

        # x staging: UNPADDED bf16 with one lead element -- elem(r, c) =
        # 1 + r*W + c.  The cast-DMA lands as one contiguous run per
        # partition; the sign and the residual matmul read it through
        # 114-wide windows at 112-row-stride (overlapping APs): the wrapped
        # edge elements produce garbage that only ever reaches discarded
        # PSUM pad columns / re-zeroed s1 pad columns.
        XU_LEN = X_ROWS * W + 2

        def load_xb(si):
            # SWDGE cast DMA: fp32 HBM -> bf16, one 128-partition transfer
            # per image pair; emitted one strip AHEAD to hide under compute
            h0i = si * R
            lo_l, hi_l = _strip_rows(si)
            xloi = lo_l + h0i - 2
            xhii = hi_l + h0i - 2
            tiles = []
            for gi in range(ngr):
                xb_t = s1pool.tile([128, XU_LEN], BF16, tag=f"xb{gi}",
                                   name="xb")
                if not SKIP_LOAD:
                    src = x_d[
                        imgs[2 * gi] : imgs[2 * gi] + 2, :, xloi:xhii, :
                    ].rearrange("i c r w -> (i c) (r w)")
                    nc.gpsimd.dma_start(
                        out=xb_t[:, 1 + lo_l * W : 1 + hi_l * W], in_=src
                    )
                else:
                    nc.gpsimd.memset(xb_t[:, 1 + lo_l * W : 1 + hi_l * W],
                                     0.5)
                # the two border elements the overlapping windows touch
                # beyond the loaded rows
                nc.gpsimd.memset(xb_t[:, lo_l * W : lo_l * W + 1], 0.0)
                nc.gpsimd.memset(
                    xb_t[:, 1 + hi_l * W : 2 + hi_l * W], 0.0
                )
                tiles.append(xb_t)
            return tiles

        def _xwin(xb_t, rsl, r0_l, nrows):
            # overlapping window AP: row k of the result reads unpadded
            # elems [(r0_l+k)*W .. +114) = [wrap | x[r0_l+k,:] | wrap]
            v = xb_t[rsl, r0_l * W : r0_l * W + 1]
            return bass.AP(
                tensor=v.tensor, offset=v.offset,
                ap=[list(v.ap[0]), [W, nrows], [1, WP]],
            )

        def prep_s1(si, xb_ts_i):
            # s1 = sign(x + b11) as +-0.5; NON-overlapping strided rows on
            # both sides (overlapping input APs run ~15x slower on the DVE
            # -- only the PE residual uses the overlap window)
            lo_l, hi_l = _strip_rows(si)
            tiles = []
            for gi, xb_t in enumerate(xb_ts_i):
                s1_t = s1pool.tile([128, X_LEN + 4], BF16, tag=f"s1{gi}",
                                   name="s1")
                s1_r = s1_t[:, 2 : 2 + X_LEN].rearrange(
                    "p (r c) -> p r c", c=WP
                )
                xu_r = xb_t[:, 1 : 1 + X_ROWS * W].rearrange(
                    "p (r c) -> p r c", c=W
                )
                dst = s1_r[:, lo_l:hi_l, 1 : 1 + W]
                srcw = xu_r[:, lo_l:hi_l, :]
                if S1_DVE:
                    nc.vector.tensor_scalar(
                        out=dst, in0=srcw,
                        scalar1=nb11, scalar2=0.5,
                        op0=mybir.AluOpType.is_ge,
                        op1=mybir.AluOpType.subtract,
                    )
                else:
                    nc.scalar.activation(
                        out=dst, in_=srcw,
                        func=mybir.ActivationFunctionType.Sign,
                        bias=v["b11"],
                    )
                nc.gpsimd.memset(s1_r[:, :, 0:1], 0.0)
                nc.gpsimd.memset(s1_r[:, :, WP - 1 : WP], 0.0)
                nc.gpsimd.memset(s1_t[:, 0:2], 0.0)
                nc.gpsimd.memset(s1_t[:, 2 + X_LEN :], 0.0)
                if si == 0 and lo_l > 0:  # top image edge
                    nc.gpsimd.memset(s1_t[:, 2 : 2 + lo_l * WP], 0.0)
                if hi_l < X_ROWS:  # bottom image edge
                    nc.gpsimd.memset(
                        s1_t[:, 2 + hi_l * WP : 2 + X_LEN], 0.0
                    )
                tiles.append(s1_t)
            return tiles

        xb_ts = load_xb(0)
        s1_ts = prep_s1(0, xb_ts)
        for s in range(NSTRIPS):
            h0 = s * R
            c1lo, c1hi = max(h0 - 1, 0), min(h0 + R + 1, H)
            # rows computed by conv1 this strip; rows [h0-1, h0+1) are
            # carried over from the previous strip's p1 (no halo recompute)
            c1start = c1lo if s == 0 else h0 + 1

            def xloc(g):   # global row -> local row in x/s1 strip
                return g - (h0 - 2)

            def ploc(g):   # global row -> local row in p1/s2 strip
                return g - (h0 - 1)

            xb_next = load_xb(s + 1) if s + 1 < NSTRIPS else None

            # -- conv1 + fused residual/scale/bias/prelu -----------------
            # p1_AB = [p1_A(lo); p1_B(hi)]; p1_CD = [p1_D(lo); p1_C(hi)]
            # bf16: feeds Sign + the conv2 residual matmul (1 cyc/row)
            p1_ts = [
                p1pool.tile([128, P_LEN + 4], BF16, tag=f"p1{gi}", name="p1")
                for gi in range(ngr)
            ]

            # carry rows [h0-1, h0+1) of p1 from the previous strip
            if s > 0:
                for gi in range(ngr):
                    nc.vector.tensor_copy(
                        out=p1_ts[gi][:, 2 : 2 + 2 * WP],
                        in_=prev_p1_ts[gi][:, 2 + R * WP : 2 + (R + 2) * WP],
                    )

            for r0, nr in _row_chunks(c1start, c1hi, step=8):
                soff_of = (
                    lambda rsl, t, _r0=r0: 2
                    + (xloc(_r0) + t // 3 - 1) * WP + (t % 3 - 1)
                )
                r0_l = xloc(r0)

                def res1_of(gi):  # unpadded x through the overlap window
                    return lambda rsl, dr, snr, _g=gi: _xwin(
                        xb_ts[_g], rsl, r0_l + dr, snr
                    )

                o1 = slice(2 + ploc(r0) * WP, 2 + ploc(r0) * WP + nr * WP)
                quads = [
                    # imgA: aligned lo->lo (pair-tile 0 lo)
                    (LO, LO, s1_ts[0], res1_of(0)),
                    # imgB: aligned hi->hi (pair-tile 0 hi)
                    (HI, HI, s1_ts[0], res1_of(0)),
                ]
                pair_outs = [p1_ts[0][:, o1]]
                if nquad:
                    quads += [
                        # imgC: crossed lo->hi (home flips to hi for conv2)
                        (LO, HI, s1_ts[1], res1_of(1)),
                        # imgD: crossed hi->lo
                        (HI, LO, s1_ts[1], res1_of(1)),
                    ]
                    pair_outs.append(p1_ts[1][:, o1])
                _conv_quad(nc, ps, w1, id1, quads, pair_outs, soff_of, nr,
                           as1, v["b12"], v["a1"])

            # -- s2 = sign(p1 + b13 + b21), zero padding -----------------
            # signed in chunk-sized pieces so conv2 chunks can start as
            # soon as their input rows are evicted (no whole-strip barrier)
            s2_pieces = ([(ploc(h0 - 1), 2)] if s > 0 else []) + [
                (ploc(r0), nr) for r0, nr in _row_chunks(c1start, c1hi, step=8)
            ]
            s2_ts = []
            for gi, p1_t in enumerate(p1_ts):
                s2_t = s2pool.tile([128, P_LEN + 4], BF16, tag=f"s2{gi}",
                                   name="s2")
                s2_r = s2_t[:, 2 : 2 + P_LEN].rearrange(
                    "p (r c) -> p r c", c=WP
                )
                for pr, pn in s2_pieces:
                    if S2_DVE:
                        nc.vector.tensor_scalar(
                            out=s2_t[:, 2 + pr * WP : 2 + (pr + pn) * WP],
                            in0=p1_t[:, 2 + pr * WP : 2 + (pr + pn) * WP],
                            scalar1=nb31, scalar2=0.5,
                            op0=mybir.AluOpType.is_ge,
                            op1=mybir.AluOpType.subtract,
                        )
                    else:
                        nc.scalar.activation(
                            out=s2_t[:, 2 + pr * WP : 2 + (pr + pn) * WP],
                            in_=p1_t[:, 2 + pr * WP : 2 + (pr + pn) * WP],
                            func=mybir.ActivationFunctionType.Sign,
                            bias=b31,
                        )
                # the sign writes full rows incl. pad cols: re-zero each strip
                nc.gpsimd.memset(s2_r[:, :, 0:1], 0.0)
                nc.gpsimd.memset(s2_r[:, :, WP - 1 : WP], 0.0)
                nc.gpsimd.memset(s2_t[:, 0:2], 0.0)
                nc.gpsimd.memset(s2_t[:, 2 + P_LEN :], 0.0)
                if ploc(c1lo) > 0:
                    nc.gpsimd.memset(s2_t[:, 2 : 2 + ploc(c1lo) * WP], 0.0)
                if ploc(c1hi) < P_ROWS:
                    nc.gpsimd.memset(
                        s2_t[:, 2 + ploc(c1hi) * WP : 2 + P_LEN], 0.0
                    )
                s2_ts.append(s2_t)

            # next strip's sign, pipelined mid-strip: the DVE runs it after
            # this strip's s2 pieces, well before conv1(s+1) needs it
            s1_next = prep_s1(s + 1, xb_next) if s + 1 < NSTRIPS else None

            # -- conv2 + fused chain -------------------------------------
            # p2_AB = [A(lo); B(hi)]; p2_CD = [C(lo); D(hi)] (crossed back)
            # unpadded bf16: evictions strip pad cols; output cast to f32
            # on the host
            p2_ts = [
                p2pool.tile([128, R * W], BF16, tag=f"p2{gi}", name="p2")
                for gi in range(ngr)
            ]
            p2_rs = [t.rearrange("p (r c) -> p r c", c=W) for t in p2_ts]
            for r0, nr in _row_chunks(h0, h0 + R, step=8):
                soff_of = (
                    lambda rsl, t, _r0=r0: 2
                    + (ploc(_r0) + t // 3 - 1) * WP + (t % 3 - 1)
                )
                roff = 2 + ploc(r0) * WP

                def res2_of(gi):  # padded p1, flat slice
                    return lambda rsl, dr, snr, _g=gi: p1_ts[_g][
                        rsl, roff + dr * WP : roff + (dr + snr) * WP
                    ]

                r2 = slice(r0 - h0, r0 - h0 + nr)
                quads = [
                    (LO, LO, s2_ts[0], res2_of(0)),
                    (HI, HI, s2_ts[0], res2_of(0)),
                ]
                pair_outs = [p2_rs[0][:, r2, :]]
                if nquad:
                    quads += [
                        # imgC now lives on hi; crossed hi->lo back home
                        (HI, LO, s2_ts[1], res2_of(1)),
                        # imgD on lo; crossed lo->hi
                        (LO, HI, s2_ts[1], res2_of(1)),
                    ]
                    pair_outs.append(p2_rs[1][:, r2, :])
                _conv_quad(nc, ps, w2, id2, quads, pair_outs, soff_of, nr,
                           as2, b32, v["a2"], strip_pads=True)

                # out2 = p2 + b23, per chunk (flat contiguous bf16 slice
                # for the 4x DVE mode)
                b23_eng = nc.gpsimd if B23_GPSIMD else nc.vector
                fs = slice((r0 - h0) * W, (r0 - h0 + nr) * W)
                for gi in range(ngr):
                    b23_eng.tensor_scalar_add(
                        p2_ts[gi][:, fs], p2_ts[gi][:, fs], v["b23"]
                    )

            # -- store: one merged 128-partition HWDGE DMA per pair ------
            for gi in range(ngr):
                if not SKIP_STORE:
                    dst = out_d[
                        imgs[2 * gi] : imgs[2 * gi] + 2, :, h0 : h0 + R, :
                    ].rearrange("i c r w -> (i c) r w")
                    nc.sync.dma_start(out=dst, in_=p2_rs[gi])
            prev_p1_ts = p1_ts
            xb_ts = xb_next
            s1_ts = s1_next


_NC_CACHE = {}


def _get_program(bl=BL):
    if bl not in _NC_CACHE:
        _NC_CACHE[bl] = build_program(bl)
    return _NC_CACHE[bl]


def make_in_maps(inputs):
    x = np.ascontiguousarray(np.asarray(inputs["x"], dtype=np.float32))
    shared = {
        "w3": np.ascontiguousarray(np.asarray(inputs["w3"], np.float32)),
        "w_pw": np.ascontiguousarray(np.asarray(inputs["w_pw"], np.float32)),
    }
    for n in WVEC_NAMES:
        shared[n] = np.ascontiguousarray(np.asarray(inputs[n], np.float32))
    return [{"x": x[i * BL : (i + 1) * BL], **shared} for i in range(NCORES)]


def run(inputs, trace=False, **kwargs):
    nc = _get_program(BL)
    res = run_bass_kernel_spmd(
        nc, make_in_maps(inputs), core_ids=list(range(NCORES)), trace=trace,
        **kwargs,
    )
    out = np.concatenate(
        [np.asarray(r["out"], dtype=np.float32) for r in res.results], axis=0
    )
    return out, res


def kernel(**inputs):
    return run(inputs)[0]


def bench(inputs, iters=20, nc=None):
    """Steady-state wall-clock benchmark: sharded jit without donation,
    device-resident inputs, async dispatch of `iters` executions."""
    import time
    import jax
    from jax.sharding import Mesh, PartitionSpec, NamedSharding
    from jax.experimental.shard_map import shard_map
    from concourse import bass2jax as b2j

    b2j.install_neuronx_cc_hook()
    if nc is None:
        nc = _get_program(BL)
    in_maps = make_in_maps(inputs)

    in_names, out_names, out_avals = [], [], []
    for alloc in nc.m.functions[0].allocations:
        if not isinstance(mybir.MemoryLocationSet, type) or not isinstance(
            alloc, mybir.MemoryLocationSet
        ):
            continue
        name = alloc.memorylocations[0].name
        if alloc.kind == "ExternalInput":
            if nc.partition_id_tensor and name == nc.partition_id_tensor.name:
                continue
            in_names.append(name)
        elif alloc.kind == "ExternalOutput":
            out_names.append(name)
            out_avals.append(
                jax.core.ShapedArray(
                    tuple(alloc.tensor_shape), mybir.dt.np(alloc.dtype)
                )
            )
    n_params = len(in_names)
    all_names = in_names + out_names
    if nc.partition_id_tensor:
        all_names = all_names + [nc.partition_id_tensor.name]

    def _body(*args):
        operands = list(args)
        if nc.partition_id_tensor:
            operands.append(b2j.partition_id_tensor())
        outs = b2j._bass_exec_p.bind(
            *operands,
            out_avals=tuple(out_avals),
            in_names=tuple(all_names),
            out_names=tuple(out_names),
            lowering_input_output_aliases=(),
            sim_require_finite=True,
            sim_require_nnan=True,
            nc=nc,
        )
        return tuple(outs)

    devices = jax.devices()[:NCORES]
    mesh = Mesh(np.asarray(devices), ("core",))
    nin = n_params + len(out_names)
    f = jax.jit(
        shard_map(
            _body,
            mesh=mesh,
            in_specs=(PartitionSpec("core"),) * nin,
            out_specs=(PartitionSpec("core"),) * len(out_names),
            check_rep=False,
        ),
        keep_unused=True,
    )
    sh = NamedSharding(mesh, PartitionSpec("core"))
    concat_in = [
        jax.device_put(np.concatenate([m[n] for m in in_maps], axis=0), sh)
        for n in in_names
    ]
    zeros = [
        jax.device_put(
            np.zeros((NCORES * a.shape[0], *a.shape[1:]), a.dtype), sh
        )
        for a in out_avals
    ]

    r = f(*concat_in, *zeros)  # warm-up / compile
    jax.block_until_ready(r)

    ts = []
    for _ in range(max(iters, 8)):
        t0 = time.perf_counter()
        r = f(*concat_in, *zeros)
        jax.block_until_ready(r)
        ts.append(time.perf_counter() - t0)
    return {"single_s": min(ts), "all": ts}


def bench_device(inputs, loops=(64, 1024), calls=10):
    """Per-iteration device time via on-device For_i repetition.  The two
    loop-count programs are dispatched in interleaved alternation so slow
    drift in dispatch overhead cancels out of the slope."""
    import time
    import jax
    from jax.sharding import Mesh, PartitionSpec, NamedSharding

    fns = {}
    for L in loops:
        nc = build_program(BL, loop_n=L)
        fns[L] = _bench_fn(inputs, nc)
    ts = {L: [] for L in loops}
    for L in loops:  # warm-up / compile
        jax.block_until_ready(fns[L]())
    for _ in range(calls):
        for L in loops:
            t0 = time.perf_counter()
            jax.block_until_ready(fns[L]())
            ts[L].append(time.perf_counter() - t0)
    res = {L: min(v) for L, v in ts.items()}
    for L in loops:
        print(f"  loop_n={L}: best single call {res[L] * 1e3:.2f} ms")
    l0, l1 = loops
    per_iter = (res[l1] - res[l0]) / (l1 - l0)
    return {"per_iter_s": per_iter, "times": res}


def _bench_fn(inputs, nc):
    """Build a zero-copy dispatch closure for `nc` (device-resident args)."""
    import jax
    from jax.sharding import Mesh, PartitionSpec, NamedSharding
    from jax.experimental.shard_map import shard_map
    from concourse import bass2jax as b2j

    b2j.install_neuronx_cc_hook()
    in_maps = make_in_maps(inputs)
    in_names, out_names, out_avals = [], [], []
    for alloc in nc.m.functions[0].allocations:
        if not isinstance(alloc, mybir.MemoryLocationSet):
            continue
        name = alloc.memorylocations[0].name
        if alloc.kind == "ExternalInput":
            if nc.partition_id_tensor and name == nc.partition_id_tensor.name:
                continue
            in_names.append(name)
        elif alloc.kind == "ExternalOutput":
            out_names.append(name)
            out_avals.append(
                jax.core.ShapedArray(
                    tuple(alloc.tensor_shape), mybir.dt.np(alloc.dtype)
                )
            )
    all_names = in_names + out_names
    if nc.partition_id_tensor:
        all_names = all_names + [nc.partition_id_tensor.name]

    def _body(*args):
        operands = list(args)
        if nc.partition_id_tensor:
            operands.append(b2j.partition_id_tensor())
        return tuple(
            b2j._bass_exec_p.bind(
                *operands,
                out_avals=tuple(out_avals),
                in_names=tuple(all_names),
                out_names=tuple(out_names),
                lowering_input_output_aliases=(),
                sim_require_finite=True,
                sim_require_nnan=True,
                nc=nc,
            )
        )

    devices = jax.devices()[:NCORES]
    mesh = Mesh(np.asarray(devices), ("core",))
    nin = len(in_names) + len(out_names)
    f = jax.jit(
        shard_map(
            _body, mesh=mesh,
            in_specs=(PartitionSpec("core"),) * nin,
            out_specs=(PartitionSpec("core"),) * len(out_names),
            check_rep=False,
        ),
        keep_unused=True,
    )
    sh = NamedSharding(mesh, PartitionSpec("core"))
    concat_in = [
        jax.device_put(np.concatenate([m[n] for m in in_maps], axis=0), sh)
        for n in in_names
    ]
    zeros = [
        jax.device_put(
            np.zeros((NCORES * a.shape[0], *a.shape[1:]), a.dtype), sh
        )
        for a in out_avals
    ]
    return lambda: f(*concat_in, *zeros)


if __name__ == "__main__":
    rng = np.random.default_rng(0)
    ins = {"x": rng.standard_normal((B, C, H, W)).astype(np.float32)}
    for n in ["w3", "w_pw"]:
        ins[n] = ((rng.random((C, C, 3, 3)) - 0.5) * 0.002).astype(np.float32)
    for n in WVEC_NAMES:
        ins[n] = (rng.standard_normal(C) * 0.01).astype(np.float32)
    out = kernel(**ins)
    print(out.shape, out.dtype)



# revision 37
# speedup vs baseline: 3.1359x; 1.1273x over previous
"""Trainium2 Bass kernel for a ReActNet-style binary BasicBlock.

Reference math per block (twice, with different weights):
    s   = sign(x + b_in)                      # +-1
    c   = conv3x3(s, mean|w| * sign(w))       # binarized conv, pad=1
    y   = x + ALPHA * c                       # residual
    y   = prelu(y + b_mid, a) + b_out

Key facts exploited:
  * matmul inputs are exactly +-1 -> bf16 matmuls are EXACT (integer sums
    accumulated in fp32 PSUM).
  * per-output-channel weight scale factors out:  conv(s, scale*sign(w)) =
    scale .* conv(s, sign(w)).
  * residual rides through PSUM via a bf16 matmul with diag(1/(ALPHA*scale))
    (bf16 streams 1 cycle/row on the PE; fp32 costs 4): x and p1 are held /
    evicted as bf16 (~0.4% rounding, well inside the 2e-2 gate).
        T = binconv(s) + x / as           (as = ALPHA*scale, per channel)
    then prelu(x + as*binconv + b, a) = Prelu-activation(T) with
    per-partition scale=as, bias=b, alpha=a  -- a single ScalarE op reading
    PSUM directly.  (prelu positive homogeneity: as > 0.)

Layout (q4i scheme): NCHW, channels (64) on partitions; FOUR images in
flight on the four 64x64 PE tiles, PAIRED into two PSUM banks (diagonal
row-split pairs write complementary partition halves of one bank; the
per-partition has_written zero-regions keep the accumulation groups
independent):
    imgA: tile(0,0)   rows 0-63  -> bankX 0-63    (aligned)
    imgB: tile(64,64) rows 64-127-> bankX 64-127  (aligned)
    imgC: tile(0,64) / tile(64,0) alternating per conv -> bankY (crossed)
    imgD: tile(64,0) / tile(0,64) alternating per conv -> bankY (crossed)
The crossed tiles flip C/D's partition home after each conv; all engine ops
stay partition-aligned, DMA handles the final placement for free.
Bank pairing lets every PSUM eviction run as ONE 128-partition ScalarE
Prelu op per pair (vs two 64-partition ops): half the ACT instructions at
full lane utilization.

Engine balance: signs run on the DVE as single tensor_scalar ops
producing +-0.5 ((x >= -b) - 0.5); the missing x2 is folded into the
per-channel eviction scale (asc = 2*ALPHA*mean|w|), keeping matmuls
exact.  Store DMAs issue from the GpSimd queue, loads from Sync, so the
ScalarE queue does nothing but the two fused Prelu evictions per chunk.

Spatial strips of R=16 output rows.  x is DMA'd into contiguous unpadded
staging (8.9KB descriptor runs, vs 448B for a padded layout) and restrided
to the padded-114 conv layout by the Sign / bf16-cast ops whose in/out APs
differ anyway; p2 is evicted unpadded for the same reason on the store
side.  conv1 rows [h0-1,h0+1) are carried over from the previous strip's
p1 (no halo recompute); s2 is signed in chunk-sized pieces so conv2 can
start before the whole strip is evicted.

Sharding: pure data parallel, batch 32 -> 4 images x 8 cores, weights
replicated, no collectives.
"""

import numpy as np
from contextlib import ExitStack, nullcontext

import concourse.bass as bass
import concourse.tile as tile
from concourse import mybir
from concourse import bacc
from concourse.bass_utils import run_bass_kernel_spmd
from concourse.masks import make_identity

B, C, H, W = 32, 64, 112, 112
ALPHA = 0.25
NCORES = 8
BL = B // NCORES          # images per core
WP = W + 2                # padded width
R = 16                    # output rows per strip
NSTRIPS = H // R

F32 = mybir.dt.float32
BF16 = mybir.dt.bfloat16

WVEC_NAMES = ["b11", "b12", "b13", "b21", "b22", "b23", "a1", "a2"]

SKIP_STORE = False   # timing experiment: drop output DMA
SKIP_LOAD = False    # timing experiment: drop input DMA (garbage data)
RESID_LAST = True    # residual matmul in slot 9 (vs slot 0)
P2_BUFS = 2          # p2 double/triple buffering
S1_DVE = True        # s1 sign on DVE (else ScalarE Sign, +-1 w/ asc1 x1)
S2_DVE = True        # s2 sign on DVE (else ScalarE Sign)
B23_GPSIMD = False   # final +b23 on GpSimd (Pool tensor_scalar: ~15x
                     # slower than DVE -- measured 13us/op; keep False)


def _bcast_ap(dram_ap, reps=2):
    """Source AP replicating a DRAM tensor across partition groups."""
    return bass.AP(
        tensor=dram_ap.tensor,
        offset=dram_ap.offset,
        ap=[[0, reps]] + [list(d) for d in dram_ap.ap],
    )


def _row_chunks(lo, hi, step=4):
    r = lo
    while r < hi:
        yield r, min(step, hi - r)
        r += step


def build_program(bl=BL, loop_n=None):
    """Build the Bass program for one core processing `bl` images.

    loop_n: if set, repeat the whole main loop on-device that many times
    (timing harness only -- results identical, just recomputed).
    """
    nc = bacc.Bacc("TRN2", target_bir_lowering=False, debug=False)

    x_d = nc.dram_tensor("x", [bl, C, H, W], F32, kind="ExternalInput").ap()
    w3_d = nc.dram_tensor("w3", [C, C, 3, 3], F32, kind="ExternalInput").ap()
    wpw_d = nc.dram_tensor("w_pw", [C, C, 3, 3], F32, kind="ExternalInput").ap()
    vec_d = {
        n: nc.dram_tensor(n, [C], F32, kind="ExternalInput").ap()
        for n in WVEC_NAMES
    }
    # bf16 output (host casts back to f32): halves the store traffic and
    # stays well inside the 2e-2 gate (adds <= 2^-9 relative rounding)
    out_d = nc.dram_tensor("out", [bl, C, H, W], BF16, kind="ExternalOutput").ap()

    with tile.TileContext(nc) as tc:
        _kernel_body(tc, out_d, x_d, w3_d, wpw_d, vec_d, bl, loop_n=loop_n)

    nc.compile()
    return nc


def _prep_conv_consts(nc, const, wdram, name, samp=1.0):
    """Per-conv constants: binarized-transposed weights, the eviction scale
    asc = ALPHA*mean|w|/samp (samp = sign amplitude: 0.5 when the DVE sign
    produces +-0.5), diag(1/asc) for the residual matmul.  Everything
    replicated on both partition halves."""
    # natural layout [co, ci*3*3] duplicated -> per-channel scale
    wn = const.tile([128, C * 9], F32, name=f"wn_{name}")
    nc.sync.dma_start(out=wn, in_=_bcast_ap(wdram.rearrange("a b c d -> a (b c d)")))
    wabs = const.tile([128, C * 9], F32, name=f"wabs_{name}")
    asum = const.tile([128, 1], F32, name=f"asum_{name}")
    nc.scalar.activation(
        out=wabs, in_=wn, func=mybir.ActivationFunctionType.Abs, accum_out=asum
    )
    asc = const.tile([128, 1], F32, name=f"asc_{name}")
    nc.vector.tensor_scalar_mul(asc, asum, ALPHA / (C * 9) / samp)
    inv_asc = const.tile([128, 1], F32, name=f"iasc_{name}")
    nc.vector.reciprocal(inv_asc, asc)

    # gathered+transposed weights [ci(+dup), co, tap], then binarize to bf16
    wg = const.tile([128, C, 9], F32, name=f"wg_{name}")
    src = bass.AP(
        tensor=wdram.tensor,
        offset=wdram.offset,
        ap=[[9, C], [C * 9, C], [1, 9]],
    )
    for rep in range(2):
        nc.sync.dma_start(out=wg[64 * rep : 64 * rep + 64, :, :], in_=src)
    wsign = const.tile([128, C, 9], BF16, name=f"ws_{name}")
    nc.scalar.activation(out=wsign, in_=wg, func=mybir.ActivationFunctionType.Sign)

    # residual injector: diag(1/as) bf16 (bf16 matmul = 1 cyc/row vs 4 for
    # fp32), per partition half
    ident = const.tile([128, C], BF16, name=f"id_{name}")
    make_identity(nc, ident[0:64, :])
    make_identity(nc, ident[64:128, :])
    nc.vector.tensor_scalar_mul(ident, ident, inv_asc)
    return wsign, asc, ident


LO = slice(0, 64)
HI = slice(64, 128)


def _conv_quad(nc, ps, w, ident, quads, pair_outs, soff_of, nr, asc, bias_mid,
               alpha, strip_pads=False):
    """One double-chunk (up to 8 output rows) of conv for four images on the
    four 64x64 PE tiles.

    quads: list of 4 tuples (rsl, osl, s_t, res_of):
      rsl: SBUF row half this image's data lives on (slice)
      osl: PSUM partition half this image's results land on (slice)
      s_t: sign tile; res_of(rsl, dr, snr): residual rhs AP for a sub-chunk
    Images 0,1 share a PSUM tile ([128, 1024] = 2 banks) and images 2,3 a
    second one (complementary partition halves; per-partition has_written
    keeps the accumulation groups independent, and the diagonal tile pairs
    stream concurrently).  Rows split into two <=4-row sub-chunks, one per
    bank (cols 0:456 and 512:968), so a full 8-row double-chunk evicts as
    ONE 128-partition ACT per pair reading both banks.
    Slots: the 9 taps then the residual-injector matmul.
    Eviction: out = Prelu(asc*psum + bias) per pair.
    """
    npair = (len(quads) + 1) // 2
    pts = [ps.tile([128, 1024], F32, tag="ps", name="pt") for _ in range(npair)]
    subs = [(0, min(4, nr))] + ([(4, nr - 4)] if nr > 4 else [])
    rslot = 9 if RESID_LAST else 0
    for s in range(10):
        for qi, (rsl, osl, s_t, res_of) in enumerate(quads):
            pt = pts[qi // 2]
            for dr, snr in subs:
                nn = snr * WP
                pc = 0 if dr == 0 else 512
                if s == rslot:
                    # residual slot: taps elsewhere need only s_t
                    nc.tensor.matmul(
                        pt[osl, pc : pc + nn], ident[rsl, :],
                        res_of(rsl, dr, snr),
                        start=(s == 0), stop=(s == 9), skip_group_check=True,
                    )
                else:
                    t = s - 1 if s > rslot else s
                    so = soff_of(rsl, t) + dr * WP
                    nc.tensor.matmul(
                        pt[osl, pc : pc + nn], w[rsl, :, t],
                        s_t[rsl, so : so + nn],
                        start=(s == 0), stop=(s == 9), skip_group_check=True,
                    )
    for pt, out_ap in zip(pts, pair_outs):
        # out_ap covers the pair's full nr rows: flat [128, nr*WP] (conv1,
        # padded) or [128, nr, W] (conv2, pads stripped)
        if nr == 8:  # one ACT reading both banks
            if strip_pads:
                src = bass.AP(
                    tensor=pt.tensor, offset=pt.offset + 1,
                    ap=[list(pt.ap[0]), [512, 2], [WP, 4], [1, W]],
                )
            else:
                src = bass.AP(
                    tensor=pt.tensor, offset=pt.offset,
                    ap=[list(pt.ap[0]), [512, 2], [1, 456]],
                )
            nc.scalar.activation(
                out=out_ap, in_=src,
                func=mybir.ActivationFunctionType.Prelu,
                bias=bias_mid, scale=asc, alpha=alpha,
            )
        else:
            for dr, snr in subs:
                pc = 0 if dr == 0 else 512
                src = pt[:, pc : pc + snr * WP]
                if strip_pads:
                    src = src.rearrange("p (r c) -> p r c", c=WP)[:, :, 1 : 1 + W]
                    dst = out_ap[:, dr : dr + snr, :]
                else:
                    dst = out_ap[:, dr * WP : (dr + snr) * WP]
                nc.scalar.activation(
                    out=dst, in_=src,
                    func=mybir.ActivationFunctionType.Prelu,
                    bias=bias_mid, scale=asc, alpha=alpha,
                )


def _kernel_body(tc, out_d, x_d, w3_d, wpw_d, vec_d, bl, loop_n=None):
    nc = tc.nc
    ctx = ExitStack()
    with ctx:
        const = ctx.enter_context(tc.tile_pool(name="const", bufs=1))
        s1pool = ctx.enter_context(tc.tile_pool(name="s1pool", bufs=2))
        p1pool = ctx.enter_context(tc.tile_pool(name="p1pool", bufs=2))
        s2pool = ctx.enter_context(tc.tile_pool(name="s2pool", bufs=2))
        p2pool = ctx.enter_context(tc.tile_pool(name="p2pool", bufs=P2_BUFS))
        # [128, 1024] psum tiles = 2 banks each; 4 bufs = all 8 banks
        ps = ctx.enter_context(tc.tile_pool(name="ps", bufs=4, space="PSUM"))

        # ---- constants -------------------------------------------------
        v = {}
        for n in WVEC_NAMES:
            v[n] = const.tile([128, 1], F32, name=f"v_{n}")
            nc.sync.dma_start(out=v[n], in_=_bcast_ap(vec_d[n]))
        b31 = const.tile([128, 1], F32, name="b31")  # b13 + b21
        nc.vector.tensor_tensor(
            out=b31, in0=v["b13"], in1=v["b21"], op=mybir.AluOpType.add
        )
        b32 = const.tile([128, 1], F32, name="b32")  # b13 + b22
        nc.vector.tensor_tensor(
            out=b32, in0=v["b13"], in1=v["b22"], op=mybir.AluOpType.add
        )
        # DVE signs compare against the negated bias: s = (u >= -b) - 0.5
        nb31 = const.tile([128, 1], F32, name="nb31")
        nc.vector.tensor_scalar_mul(nb31, b31, -1.0)
        nb11 = const.tile([128, 1], F32, name="nb11")
        nc.vector.tensor_scalar_mul(nb11, v["b11"], -1.0)

        w1, as1, id1 = _prep_conv_consts(nc, const, w3_d, "c1",
                                         samp=0.5 if S1_DVE else 1.0)
        w2, as2, id2 = _prep_conv_consts(nc, const, wpw_d, "c2",
                                         samp=0.5 if S2_DVE else 1.0)

        loop_cm = tc.For_i(0, loop_n, 1) if loop_n else nullcontext()
        with loop_cm:
            _main_strips(tc, nc, out_d, x_d, bl, v, b31, b32, nb11, nb31,
                         w1, as1, id1, w2, as2, id2,
                         s1pool, p1pool, s2pool, p2pool, ps)


def _main_strips(tc, nc, out_d, x_d, bl, v, b31, b32, nb11, nb31,
                 w1, as1, id1, w2, as2, id2,
                 s1pool, p1pool, s2pool, p2pool, ps):
    X_ROWS = R + 4     # x / s1 strip rows   [h0-2, h0+R+2)
    P_ROWS = R + 2     # p1 / s2 strip rows  [h0-1, h0+R+1)
    X_LEN = X_ROWS * WP
    P_LEN = P_ROWS * WP

    # groups of 4 images (quad) or 2 (pair, AB tiles only — sim harness)
    groups = []
    i = 0
    while i < bl:
        g = min(4, bl - i)
        assert g in (2, 4)
        groups.append(list(range(i, i + g)))
        i += g

    for imgs in groups:
        nquad = len(imgs) == 4
        ngr = len(imgs) // 2
        prev_p1_ts = None

        def _strip_rows(si):
            h0i = si * R
            c1s = max(h0i - 1, 0) if si == 0 else h0i + 1
            c1h = min(h0i + R + 1, H)
            lo_l = max(c1s - 1, 0) - (h0i - 2)
            hi_l = min(c1h + 1, H) - (h0i - 2)
            return lo_l, hi_l

        # x staging: UNPADDED bf16 with one lead element -- elem(r, c) =
        # 1 + r*W + c.  The cast-DMA lands as one contiguous run per
        # partition; the sign and the residual matmul read it through
        # 114-wide windows at 112-row-stride (overlapping APs): the wrapped
        # edge elements produce garbage that only ever reaches discarded
        # PSUM pad columns / re-zeroed s1 pad columns.
        XU_LEN = X_ROWS * W + 2

        def load_xb(si):
            # SWDGE cast DMA: fp32 HBM -> bf16, one 128-partition transfer
            # per image pair; emitted one strip AHEAD to hide under compute
            h0i = si * R
            lo_l, hi_l = _strip_rows(si)
            xloi = lo_l + h0i - 2
            xhii = hi_l + h0i - 2
            tiles = []
            for gi in range(ngr):
                xb_t = s1pool.tile([128, XU_LEN], BF16, tag=f"xb{gi}",
                                   name="xb")
                if not SKIP_LOAD:
                    src = x_d[
                        imgs[2 * gi] : imgs[2 * gi] + 2, :, xloi:xhii, :
                    ].rearrange("i c r w -> (i c) (r w)")
                    nc.gpsimd.dma_start(
                        out=xb_t[:, 1 + lo_l * W : 1 + hi_l * W], in_=src
                    )
                else:
                    nc.gpsimd.memset(xb_t[:, 1 + lo_l * W : 1 + hi_l * W],
                                     0.5)
                # the two border elements the overlapping windows touch
                # beyond the loaded rows
                nc.gpsimd.memset(xb_t[:, lo_l * W : lo_l * W + 1], 0.0)
                nc.gpsimd.memset(
                    xb_t[:, 1 + hi_l * W : 2 + hi_l * W], 0.0
                )
                tiles.append(xb_t)
            return tiles

        def _xwin(xb_t, rsl, r0_l, nrows):
            # overlapping window AP: row k of the result reads unpadded
            # elems [(r0_l+k)*W .. +114) = [wrap | x[r0_l+k,:] | wrap]
            v = xb_t[rsl, r0_l * W : r0_l * W + 1]
            return bass.AP(
                tensor=v.tensor, offset=v.offset,
                ap=[list(v.ap[0]), [W, nrows], [1, WP]],
            )

        def prep_s1(si, xb_ts_i):
            # s1 = sign(x + b11) as +-0.5; NON-overlapping strided rows on
            # both sides (overlapping input APs run ~15x slower on the DVE
            # -- only the PE residual uses the overlap window)
            lo_l, hi_l = _strip_rows(si)
            tiles = []
            for gi, xb_t in enumerate(xb_ts_i):
                s1_t = s1pool.tile([128, X_LEN + 4], BF16, tag=f"s1{gi}",
                                   name="s1")
                s1_r = s1_t[:, 2 : 2 + X_LEN].rearrange(
                    "p (r c) -> p r c", c=WP
                )
                xu_r = xb_t[:, 1 : 1 + X_ROWS * W].rearrange(
                    "p (r c) -> p r c", c=W
                )
                dst = s1_r[:, lo_l:hi_l, 1 : 1 + W]
                srcw = xu_r[:, lo_l:hi_l, :]
                if S1_DVE:
                    nc.vector.tensor_scalar(
                        out=dst, in0=srcw,
                        scalar1=nb11, scalar2=0.5,
                        op0=mybir.AluOpType.is_ge,
                        op1=mybir.AluOpType.subtract,
                    )
                else:
                    nc.scalar.activation(
                        out=dst, in_=srcw,
                        func=mybir.ActivationFunctionType.Sign,
                        bias=v["b11"],
                    )
                nc.gpsimd.memset(s1_r[:, :, 0:1], 0.0)
                nc.gpsimd.memset(s1_r[:, :, WP - 1 : WP], 0.0)
                nc.gpsimd.memset(s1_t[:, 0:2], 0.0)
                nc.gpsimd.memset(s1_t[:, 2 + X_LEN :], 0.0)
                if si == 0 and lo_l > 0:  # top image edge
                    nc.gpsimd.memset(s1_t[:, 2 : 2 + lo_l * WP], 0.0)
                if hi_l < X_ROWS:  # bottom image edge
                    nc.gpsimd.memset(
                        s1_t[:, 2 + hi_l * WP : 2 + X_LEN], 0.0
                    )
                tiles.append(s1_t)
            return tiles

        xb_ts = load_xb(0)
        s1_ts = prep_s1(0, xb_ts)
        for s in range(NSTRIPS):
            h0 = s * R
            c1lo, c1hi = max(h0 - 1, 0), min(h0 + R + 1, H)
            # rows computed by conv1 this strip; rows [h0-1, h0+1) are
            # carried over from the previous strip's p1 (no halo recompute)
            c1start = c1lo if s == 0 else h0 + 1

            def xloc(g):   # global row -> local row in x/s1 strip
                return g - (h0 - 2)

            def ploc(g):   # global row -> local row in p1/s2 strip
                return g - (h0 - 1)

            xb_next = load_xb(s + 1) if s + 1 < NSTRIPS else None

            # -- conv1 + fused residual/scale/bias/prelu -----------------
            # p1_AB = [p1_A(lo); p1_B(hi)]; p1_CD = [p1_D(lo); p1_C(hi)]
            # bf16: feeds Sign + the conv2 residual matmul (1 cyc/row)
            p1_ts = [
                p1pool.tile([128, P_LEN + 4], BF16, tag=f"p1{gi}", name="p1")
                for gi in range(ngr)
            ]

            # carry rows [h0-1, h0+1) of p1 from the previous strip
            if s > 0:
                for gi in range(ngr):
                    nc.vector.tensor_copy(
                        out=p1_ts[gi][:, 2 : 2 + 2 * WP],
                        in_=prev_p1_ts[gi][:, 2 + R * WP : 2 + (R + 2) * WP],
                    )

            for r0, nr in _row_chunks(c1start, c1hi, step=8):
                soff_of = (
                    lambda rsl, t, _r0=r0: 2
                    + (xloc(_r0) + t // 3 - 1) * WP + (t % 3 - 1)
                )
                r0_l = xloc(r0)

                def res1_of(gi):  # unpadded x through the overlap window
                    return lambda rsl, dr, snr, _g=gi: _xwin(
                        xb_ts[_g], rsl, r0_l + dr, snr
                    )

                o1 = slice(2 + ploc(r0) * WP, 2 + ploc(r0) * WP + nr * WP)
                quads = [
                    # imgA: aligned lo->lo (pair-tile 0 lo)
                    (LO, LO, s1_ts[0], res1_of(0)),
                    # imgB: aligned hi->hi (pair-tile 0 hi)
                    (HI, HI, s1_ts[0], res1_of(0)),
                ]
                pair_outs = [p1_ts[0][:, o1]]
                if nquad:
                    quads += [
                        # imgC: crossed lo->hi (home flips to hi for conv2)
                        (LO, HI, s1_ts[1], res1_of(1)),
                        # imgD: crossed hi->lo
                        (HI, LO, s1_ts[1], res1_of(1)),
                    ]
                    pair_outs.append(p1_ts[1][:, o1])
                _conv_quad(nc, ps, w1, id1, quads, pair_outs, soff_of, nr,
                           as1, v["b12"], v["a1"])

            # next strip's sign, pipelined: the load has been in flight
            # since the strip top, and the s2 pieces behind it on the DVE
            # queue gate on conv1 evictions anyway
            s1_next = prep_s1(s + 1, xb_next) if s + 1 < NSTRIPS else None

            # -- s2 = sign(p1 + b13 + b21), zero padding -----------------
            # signed in chunk-sized pieces so conv2 chunks can start as
            # soon as their input rows are evicted (no whole-strip barrier)
            s2_pieces = ([(ploc(h0 - 1), 2)] if s > 0 else []) + [
                (ploc(r0), nr) for r0, nr in _row_chunks(c1start, c1hi, step=8)
            ]
            s2_ts = []
            for gi, p1_t in enumerate(p1_ts):
                s2_t = s2pool.tile([128, P_LEN + 4], BF16, tag=f"s2{gi}",
                                   name="s2")
                s2_r = s2_t[:, 2 : 2 + P_LEN].rearrange(
                    "p (r c) -> p r c", c=WP
                )
                p1_r = p1_t[:, 2 : 2 + P_LEN].rearrange(
                    "p (r c) -> p r c", c=WP
                )
                for pr, pn in s2_pieces:
                    # strided (pad cols skipped): keeps the pad memsets
                    # free of WAW deps on the sign, so they never block
                    # the GpSimd queue at strip boundaries
                    if S2_DVE:
                        nc.vector.tensor_scalar(
                            out=s2_r[:, pr : pr + pn, 1 : 1 + W],
                            in0=p1_r[:, pr : pr + pn, 1 : 1 + W],
                            scalar1=nb31, scalar2=0.5,
                            op0=mybir.AluOpType.is_ge,
                            op1=mybir.AluOpType.subtract,
                        )
                    else:
                        nc.scalar.activation(
                            out=s2_r[:, pr : pr + pn, 1 : 1 + W],
                            in_=p1_r[:, pr : pr + pn, 1 : 1 + W],
                            func=mybir.ActivationFunctionType.Sign,
                            bias=b31,
                        )
                nc.gpsimd.memset(s2_r[:, :, 0:1], 0.0)
                nc.gpsimd.memset(s2_r[:, :, WP - 1 : WP], 0.0)
                nc.gpsimd.memset(s2_t[:, 0:2], 0.0)
                nc.gpsimd.memset(s2_t[:, 2 + P_LEN :], 0.0)
                if ploc(c1lo) > 0:
                    nc.gpsimd.memset(s2_t[:, 2 : 2 + ploc(c1lo) * WP], 0.0)
                if ploc(c1hi) < P_ROWS:
                    nc.gpsimd.memset(
                        s2_t[:, 2 + ploc(c1hi) * WP : 2 + P_LEN], 0.0
                    )
                s2_ts.append(s2_t)

            # -- conv2 + fused chain -------------------------------------
            # p2_AB = [A(lo); B(hi)]; p2_CD = [C(lo); D(hi)] (crossed back)
            # unpadded bf16: evictions strip pad cols; output cast to f32
            # on the host
            p2_ts = [
                p2pool.tile([128, R * W], BF16, tag=f"p2{gi}", name="p2")
                for gi in range(ngr)
            ]
            p2_rs = [t.rearrange("p (r c) -> p r c", c=W) for t in p2_ts]
            for r0, nr in _row_chunks(h0, h0 + R, step=8):
                soff_of = (
                    lambda rsl, t, _r0=r0: 2
                    + (ploc(_r0) + t // 3 - 1) * WP + (t % 3 - 1)
                )
                roff = 2 + ploc(r0) * WP

                def res2_of(gi):  # padded p1, flat slice
                    return lambda rsl, dr, snr, _g=gi: p1_ts[_g][
                        rsl, roff + dr * WP : roff + (dr + snr) * WP
                    ]

                r2 = slice(r0 - h0, r0 - h0 + nr)
                quads = [
                    (LO, LO, s2_ts[0], res2_of(0)),
                    (HI, HI, s2_ts[0], res2_of(0)),
                ]
                pair_outs = [p2_rs[0][:, r2, :]]
                if nquad:
                    quads += [
                        # imgC now lives on hi; crossed hi->lo back home
                        (HI, LO, s2_ts[1], res2_of(1)),
                        # imgD on lo; crossed lo->hi
                        (LO, HI, s2_ts[1], res2_of(1)),
                    ]
                    pair_outs.append(p2_rs[1][:, r2, :])
                _conv_quad(nc, ps, w2, id2, quads, pair_outs, soff_of, nr,
                           as2, b32, v["a2"], strip_pads=True)

                # out2 = p2 + b23, per chunk (flat contiguous bf16 slice
                # for the 4x DVE mode)
                b23_eng = nc.gpsimd if B23_GPSIMD else nc.vector
                fs = slice((r0 - h0) * W, (r0 - h0 + nr) * W)
                for gi in range(ngr):
                    b23_eng.tensor_scalar_add(
                        p2_ts[gi][:, fs], p2_ts[gi][:, fs], v["b23"]
                    )

            # -- store: one merged 128-partition HWDGE DMA per pair ------
            for gi in range(ngr):
                if not SKIP_STORE:
                    dst = out_d[
                        imgs[2 * gi] : imgs[2 * gi] + 2, :, h0 : h0 + R, :
                    ].rearrange("i c r w -> (i c) r w")
                    nc.sync.dma_start(out=dst, in_=p2_rs[gi])
            prev_p1_ts = p1_ts
            xb_ts = xb_next
            s1_ts = s1_next


_NC_CACHE = {}


def _get_program(bl=BL):
    if bl not in _NC_CACHE:
        _NC_CACHE[bl] = build_program(bl)
    return _NC_CACHE[bl]


def make_in_maps(inputs):
    x = np.ascontiguousarray(np.asarray(inputs["x"], dtype=np.float32))
    shared = {
        "w3": np.ascontiguousarray(np.asarray(inputs["w3"], np.float32)),
        "w_pw": np.ascontiguousarray(np.asarray(inputs["w_pw"], np.float32)),
    }
    for n in WVEC_NAMES:
        shared[n] = np.ascontiguousarray(np.asarray(inputs[n], np.float32))
    return [{"x": x[i * BL : (i + 1) * BL], **shared} for i in range(NCORES)]


def run(inputs, trace=False, **kwargs):
    nc = _get_program(BL)
    res = run_bass_kernel_spmd(
        nc, make_in_maps(inputs), core_ids=list(range(NCORES)), trace=trace,
        **kwargs,
    )
    out = np.concatenate(
        [np.asarray(r["out"], dtype=np.float32) for r in res.results], axis=0
    )
    return out, res


def kernel(**inputs):
    return run(inputs)[0]


def bench(inputs, iters=20, nc=None):
    """Steady-state wall-clock benchmark: sharded jit without donation,
    device-resident inputs, async dispatch of `iters` executions."""
    import time
    import jax
    from jax.sharding import Mesh, PartitionSpec, NamedSharding
    from jax.experimental.shard_map import shard_map
    from concourse import bass2jax as b2j

    b2j.install_neuronx_cc_hook()
    if nc is None:
        nc = _get_program(BL)
    in_maps = make_in_maps(inputs)

    in_names, out_names, out_avals = [], [], []
    for alloc in nc.m.functions[0].allocations:
        if not isinstance(mybir.MemoryLocationSet, type) or not isinstance(
            alloc, mybir.MemoryLocationSet
        ):
            continue
        name = alloc.memorylocations[0].name
        if alloc.kind == "ExternalInput":
            if nc.partition_id_tensor and name == nc.partition_id_tensor.name:
                continue
            in_names.append(name)
        elif alloc.kind == "ExternalOutput":
            out_names.append(name)
            out_avals.append(
                jax.core.ShapedArray(
                    tuple(alloc.tensor_shape), mybir.dt.np(alloc.dtype)
                )
            )
    n_params = len(in_names)
    all_names = in_names + out_names
    if nc.partition_id_tensor:
        all_names = all_names + [nc.partition_id_tensor.name]

    def _body(*args):
        operands = list(args)
        if nc.partition_id_tensor:
            operands.append(b2j.partition_id_tensor())
        outs = b2j._bass_exec_p.bind(
            *operands,
            out_avals=tuple(out_avals),
            in_names=tuple(all_names),
            out_names=tuple(out_names),
            lowering_input_output_aliases=(),
            sim_require_finite=True,
            sim_require_nnan=True,
            nc=nc,
        )
        return tuple(outs)

    devices = jax.devices()[:NCORES]
    mesh = Mesh(np.asarray(devices), ("core",))
    nin = n_params + len(out_names)
    f = jax.jit(
        shard_map(
            _body,
            mesh=mesh,
            in_specs=(PartitionSpec("core"),) * nin,
            out_specs=(PartitionSpec("core"),) * len(out_names),
            check_rep=False,
        ),
        keep_unused=True,
    )
    sh = NamedSharding(mesh, PartitionSpec("core"))
    concat_in = [
        jax.device_put(np.concatenate([m[n] for m in in_maps], axis=0), sh)
        for n in in_names
    ]
    zeros = [
        jax.device_put(
            np.zeros((NCORES * a.shape[0], *a.shape[1:]), a.dtype), sh
        )
        for a in out_avals
    ]

    r = f(*concat_in, *zeros)  # warm-up / compile
    jax.block_until_ready(r)

    ts = []
    for _ in range(max(iters, 8)):
        t0 = time.perf_counter()
        r = f(*concat_in, *zeros)
        jax.block_until_ready(r)
        ts.append(time.perf_counter() - t0)
    return {"single_s": min(ts), "all": ts}


def bench_device(inputs, loops=(64, 1024), calls=10):
    """Per-iteration device time via on-device For_i repetition.  The two
    loop-count programs are dispatched in interleaved alternation so slow
    drift in dispatch overhead cancels out of the slope."""
    import time
    import jax
    from jax.sharding import Mesh, PartitionSpec, NamedSharding

    fns = {}
    for L in loops:
        nc = build_program(BL, loop_n=L)
        fns[L] = _bench_fn(inputs, nc)
    ts = {L: [] for L in loops}
    for L in loops:  # warm-up / compile
        jax.block_until_ready(fns[L]())
    for _ in range(calls):
        for L in loops:
            t0 = time.perf_counter()
            jax.block_until_ready(fns[L]())
            ts[L].append(time.perf_counter() - t0)
    res = {L: min(v) for L, v in ts.items()}
    for L in loops:
        print(f"  loop_n={L}: best single call {res[L] * 1e3:.2f} ms")
    l0, l1 = loops
    per_iter = (res[l1] - res[l0]) / (l1 - l0)
    return {"per_iter_s": per_iter, "times": res}


def _bench_fn(inputs, nc):
    """Build a zero-copy dispatch closure for `nc` (device-resident args)."""
    import jax
    from jax.sharding import Mesh, PartitionSpec, NamedSharding
    from jax.experimental.shard_map import shard_map
    from concourse import bass2jax as b2j

    b2j.install_neuronx_cc_hook()
    in_maps = make_in_maps(inputs)
    in_names, out_names, out_avals = [], [], []
    for alloc in nc.m.functions[0].allocations:
        if not isinstance(alloc, mybir.MemoryLocationSet):
            continue
        name = alloc.memorylocations[0].name
        if alloc.kind == "ExternalInput":
            if nc.partition_id_tensor and name == nc.partition_id_tensor.name:
                continue
            in_names.append(name)
        elif alloc.kind == "ExternalOutput":
            out_names.append(name)
            out_avals.append(
                jax.core.ShapedArray(
                    tuple(alloc.tensor_shape), mybir.dt.np(alloc.dtype)
                )
            )
    all_names = in_names + out_names
    if nc.partition_id_tensor:
        all_names = all_names + [nc.partition_id_tensor.name]

    def _body(*args):
        operands = list(args)
        if nc.partition_id_tensor:
            operands.append(b2j.partition_id_tensor())
        return tuple(
            b2j._bass_exec_p.bind(
                *operands,
                out_avals=tuple(out_avals),
                in_names=tuple(all_names),
                out_names=tuple(out_names),
                lowering_input_output_aliases=(),
                sim_require_finite=True,
                sim_require_nnan=True,
                nc=nc,
            )
        )

    devices = jax.devices()[:NCORES]
    mesh = Mesh(np.asarray(devices), ("core",))
    nin = len(in_names) + len(out_names)
    f = jax.jit(
        shard_map(
            _body, mesh=mesh,
            in_specs=(PartitionSpec("core"),) * nin,
            out_specs=(PartitionSpec("core"),) * len(out_names),
            check_rep=False,
        ),
        keep_unused=True,
    )
    sh = NamedSharding(mesh, PartitionSpec("core"))
    concat_in = [
        jax.device_put(np.concatenate([m[n] for m in in_maps], axis=0), sh)
        for n in in_names
    ]
    zeros = [
        jax.device_put(
            np.zeros((NCORES * a.shape[0], *a.shape[1:]), a.dtype), sh
        )
        for a in out_avals
    ]
    return lambda: f(*concat_in, *zeros)


if __name__ == "__main__":
    rng = np.random.default_rng(0)
    ins = {"x": rng.standard_normal((B, C, H, W)).astype(np.float32)}
    for n in ["w3", "w_pw"]:
        ins[n] = ((rng.random((C, C, 3, 3)) - 0.5) * 0.002).astype(np.float32)
    for n in WVEC_NAMES:
        ins[n] = (rng.standard_normal(C) * 0.01).astype(np.float32)
    out = kernel(**ins)
    print(out.shape, out.dtype)



# revision 38
# speedup vs baseline: 3.1764x; 1.0129x over previous
"""Trainium2 Bass kernel for a ReActNet-style binary BasicBlock.

Reference math per block (twice, with different weights):
    s   = sign(x + b_in)                      # +-1
    c   = conv3x3(s, mean|w| * sign(w))       # binarized conv, pad=1
    y   = x + ALPHA * c                       # residual
    y   = prelu(y + b_mid, a) + b_out

Key facts exploited:
  * matmul inputs are exactly +-1 -> bf16 matmuls are EXACT (integer sums
    accumulated in fp32 PSUM).
  * per-output-channel weight scale factors out:  conv(s, scale*sign(w)) =
    scale .* conv(s, sign(w)).
  * residual rides through PSUM via a bf16 matmul with diag(1/(ALPHA*scale))
    (bf16 streams 1 cycle/row on the PE; fp32 costs 4): x and p1 are held /
    evicted as bf16 (~0.4% rounding, well inside the 2e-2 gate).
        T = binconv(s) + x / as           (as = ALPHA*scale, per channel)
    then prelu(x + as*binconv + b, a) = Prelu-activation(T) with
    per-partition scale=as, bias=b, alpha=a  -- a single ScalarE op reading
    PSUM directly.  (prelu positive homogeneity: as > 0.)

Layout (q4i scheme): NCHW, channels (64) on partitions; FOUR images in
flight on the four 64x64 PE tiles, PAIRED into two PSUM banks (diagonal
row-split pairs write complementary partition halves of one bank; the
per-partition has_written zero-regions keep the accumulation groups
independent):
    imgA: tile(0,0)   rows 0-63  -> bankX 0-63    (aligned)
    imgB: tile(64,64) rows 64-127-> bankX 64-127  (aligned)
    imgC: tile(0,64) / tile(64,0) alternating per conv -> bankY (crossed)
    imgD: tile(64,0) / tile(0,64) alternating per conv -> bankY (crossed)
The crossed tiles flip C/D's partition home after each conv; all engine ops
stay partition-aligned, DMA handles the final placement for free.
Bank pairing lets every PSUM eviction run as ONE 128-partition ScalarE
Prelu op per pair (vs two 64-partition ops): half the ACT instructions at
full lane utilization.

Engine balance: signs run on the DVE as single tensor_scalar ops
producing +-0.5 ((x >= -b) - 0.5); the missing x2 is folded into the
per-channel eviction scale (asc = 2*ALPHA*mean|w|), keeping matmuls
exact.  Store DMAs issue from the GpSimd queue, loads from Sync, so the
ScalarE queue does nothing but the two fused Prelu evictions per chunk.

Spatial strips of R=16 output rows.  x is DMA'd into contiguous unpadded
staging (8.9KB descriptor runs, vs 448B for a padded layout) and restrided
to the padded-114 conv layout by the Sign / bf16-cast ops whose in/out APs
differ anyway; p2 is evicted unpadded for the same reason on the store
side.  conv1 rows [h0-1,h0+1) are carried over from the previous strip's
p1 (no halo recompute); s2 is signed in chunk-sized pieces so conv2 can
start before the whole strip is evicted.

Sharding: pure data parallel, batch 32 -> 4 images x 8 cores, weights
replicated, no collectives.
"""

import numpy as np
from contextlib import ExitStack, nullcontext

import concourse.bass as bass
import concourse.tile as tile
from concourse import mybir
from concourse import bacc
from concourse.bass_utils import run_bass_kernel_spmd
from concourse.masks import make_identity

B, C, H, W = 32, 64, 112, 112
ALPHA = 0.25
NCORES = 8
BL = B // NCORES          # images per core
WP = W + 2                # padded width
R = 16                    # output rows per strip
NSTRIPS = H // R

F32 = mybir.dt.float32
BF16 = mybir.dt.bfloat16

WVEC_NAMES = ["b11", "b12", "b13", "b21", "b22", "b23", "a1", "a2"]

SKIP_STORE = False   # timing experiment: drop output DMA
SKIP_LOAD = False    # timing experiment: drop input DMA (garbage data)
RESID_LAST = True    # residual matmul in slot 9 (vs slot 0)
P2_BUFS = 2          # p2 double/triple buffering
S1_DVE = True        # s1 sign on DVE (else ScalarE Sign, +-1 w/ asc1 x1)
S2_DVE = True        # s2 sign on DVE (else ScalarE Sign)
B23_GPSIMD = False   # final +b23 on GpSimd (Pool tensor_scalar: ~15x
                     # slower than DVE -- measured 13us/op; keep False)


def _bcast_ap(dram_ap, reps=2):
    """Source AP replicating a DRAM tensor across partition groups."""
    return bass.AP(
        tensor=dram_ap.tensor,
        offset=dram_ap.offset,
        ap=[[0, reps]] + [list(d) for d in dram_ap.ap],
    )


def _row_chunks(lo, hi, step=4):
    r = lo
    while r < hi:
        yield r, min(step, hi - r)
        r += step


def build_program(bl=BL, loop_n=None):
    """Build the Bass program for one core processing `bl` images.

    loop_n: if set, repeat the whole main loop on-device that many times
    (timing harness only -- results identical, just recomputed).
    """
    nc = bacc.Bacc("TRN2", target_bir_lowering=False, debug=False)

    x_d = nc.dram_tensor("x", [bl, C, H, W], F32, kind="ExternalInput").ap()
    w3_d = nc.dram_tensor("w3", [C, C, 3, 3], F32, kind="ExternalInput").ap()
    wpw_d = nc.dram_tensor("w_pw", [C, C, 3, 3], F32, kind="ExternalInput").ap()
    vec_d = {
        n: nc.dram_tensor(n, [C], F32, kind="ExternalInput").ap()
        for n in WVEC_NAMES
    }
    # bf16 output (host casts back to f32): halves the store traffic and
    # stays well inside the 2e-2 gate (adds <= 2^-9 relative rounding)
    out_d = nc.dram_tensor("out", [bl, C, H, W], BF16, kind="ExternalOutput").ap()

    with tile.TileContext(nc) as tc:
        _kernel_body(tc, out_d, x_d, w3_d, wpw_d, vec_d, bl, loop_n=loop_n)

    nc.compile()
    return nc


def _prep_conv_consts(nc, const, wdram, name, samp=1.0):
    """Per-conv constants: binarized-transposed weights, the eviction scale
    asc = ALPHA*mean|w|/samp (samp = sign amplitude: 0.5 when the DVE sign
    produces +-0.5), diag(1/asc) for the residual matmul.  Everything
    replicated on both partition halves."""
    # natural layout [co, ci*3*3] duplicated -> per-channel scale
    wn = const.tile([128, C * 9], F32, name=f"wn_{name}")
    nc.sync.dma_start(out=wn, in_=_bcast_ap(wdram.rearrange("a b c d -> a (b c d)")))
    wabs = const.tile([128, C * 9], F32, name=f"wabs_{name}")
    asum = const.tile([128, 1], F32, name=f"asum_{name}")
    nc.scalar.activation(
        out=wabs, in_=wn, func=mybir.ActivationFunctionType.Abs, accum_out=asum
    )
    asc = const.tile([128, 1], F32, name=f"asc_{name}")
    nc.vector.tensor_scalar_mul(asc, asum, ALPHA / (C * 9) / samp)
    inv_asc = const.tile([128, 1], F32, name=f"iasc_{name}")
    nc.vector.reciprocal(inv_asc, asc)

    # gathered+transposed weights [ci(+dup), co, tap], then binarize to bf16
    wg = const.tile([128, C, 9], F32, name=f"wg_{name}")
    src = bass.AP(
        tensor=wdram.tensor,
        offset=wdram.offset,
        ap=[[9, C], [C * 9, C], [1, 9]],
    )
    for rep in range(2):
        nc.sync.dma_start(out=wg[64 * rep : 64 * rep + 64, :, :], in_=src)
    wsign = const.tile([128, C, 9], BF16, name=f"ws_{name}")
    nc.scalar.activation(out=wsign, in_=wg, func=mybir.ActivationFunctionType.Sign)

    # residual injector: diag(1/as) bf16 (bf16 matmul = 1 cyc/row vs 4 for
    # fp32), per partition half
    ident = const.tile([128, C], BF16, name=f"id_{name}")
    make_identity(nc, ident[0:64, :])
    make_identity(nc, ident[64:128, :])
    nc.vector.tensor_scalar_mul(ident, ident, inv_asc)
    return wsign, asc, ident


LO = slice(0, 64)
HI = slice(64, 128)


def _conv_quad(nc, ps, w, ident, quads, pair_outs, soff_of, nr, asc, bias_mid,
               alpha, strip_pads=False):
    """One double-chunk (up to 8 output rows) of conv for four images on the
    four 64x64 PE tiles.

    quads: list of 4 tuples (rsl, osl, s_t, res_of):
      rsl: SBUF row half this image's data lives on (slice)
      osl: PSUM partition half this image's results land on (slice)
      s_t: sign tile; res_of(rsl, dr, snr): residual rhs AP for a sub-chunk
    Images 0,1 share a PSUM tile ([128, 1024] = 2 banks) and images 2,3 a
    second one (complementary partition halves; per-partition has_written
    keeps the accumulation groups independent, and the diagonal tile pairs
    stream concurrently).  Rows split into two <=4-row sub-chunks, one per
    bank (cols 0:456 and 512:968), so a full 8-row double-chunk evicts as
    ONE 128-partition ACT per pair reading both banks.
    Slots: the 9 taps then the residual-injector matmul.
    Eviction: out = Prelu(asc*psum + bias) per pair.
    """
    npair = (len(quads) + 1) // 2
    pts = [ps.tile([128, 1024], F32, tag="ps", name="pt") for _ in range(npair)]
    subs = [(0, min(4, nr))] + ([(4, nr - 4)] if nr > 4 else [])
    rslot = 9 if RESID_LAST else 0
    for s in range(10):
        for qi, (rsl, osl, s_t, res_of) in enumerate(quads):
            pt = pts[qi // 2]
            for dr, snr in subs:
                nn = snr * WP
                pc = 0 if dr == 0 else 512
                if s == rslot:
                    # residual slot: taps elsewhere need only s_t
                    nc.tensor.matmul(
                        pt[osl, pc : pc + nn], ident[rsl, :],
                        res_of(rsl, dr, snr),
                        start=(s == 0), stop=(s == 9), skip_group_check=True,
                    )
                else:
                    t = s - 1 if s > rslot else s
                    so = soff_of(rsl, t) + dr * WP
                    nc.tensor.matmul(
                        pt[osl, pc : pc + nn], w[rsl, :, t],
                        s_t[rsl, so : so + nn],
                        start=(s == 0), stop=(s == 9), skip_group_check=True,
                    )
    for pt, out_ap in zip(pts, pair_outs):
        # out_ap covers the pair's full nr rows: flat [128, nr*WP] (conv1,
        # padded) or [128, nr, W] (conv2, pads stripped)
        if nr == 8:  # one ACT reading both banks
            if strip_pads:
                src = bass.AP(
                    tensor=pt.tensor, offset=pt.offset + 1,
                    ap=[list(pt.ap[0]), [512, 2], [WP, 4], [1, W]],
                )
            else:
                src = bass.AP(
                    tensor=pt.tensor, offset=pt.offset,
                    ap=[list(pt.ap[0]), [512, 2], [1, 456]],
                )
            nc.scalar.activation(
                out=out_ap, in_=src,
                func=mybir.ActivationFunctionType.Prelu,
                bias=bias_mid, scale=asc, alpha=alpha,
            )
        else:
            for dr, snr in subs:
                pc = 0 if dr == 0 else 512
                src = pt[:, pc : pc + snr * WP]
                if strip_pads:
                    src = src.rearrange("p (r c) -> p r c", c=WP)[:, :, 1 : 1 + W]
                    dst = out_ap[:, dr : dr + snr, :]
                else:
                    dst = out_ap[:, dr * WP : (dr + snr) * WP]
                nc.scalar.activation(
                    out=dst, in_=src,
                    func=mybir.ActivationFunctionType.Prelu,
                    bias=bias_mid, scale=asc, alpha=alpha,
                )


def _kernel_body(tc, out_d, x_d, w3_d, wpw_d, vec_d, bl, loop_n=None):
    nc = tc.nc
    ctx = ExitStack()
    with ctx:
        const = ctx.enter_context(tc.tile_pool(name="const", bufs=1))
        s1pool = ctx.enter_context(tc.tile_pool(name="s1pool", bufs=2))
        p1pool = ctx.enter_context(tc.tile_pool(name="p1pool", bufs=2))
        s2pool = ctx.enter_context(tc.tile_pool(name="s2pool", bufs=2))
        p2pool = ctx.enter_context(tc.tile_pool(name="p2pool", bufs=P2_BUFS))
        # [128, 1024] psum tiles = 2 banks each; 4 bufs = all 8 banks
        ps = ctx.enter_context(tc.tile_pool(name="ps", bufs=4, space="PSUM"))

        # ---- constants -------------------------------------------------
        v = {}
        for n in WVEC_NAMES:
            v[n] = const.tile([128, 1], F32, name=f"v_{n}")
            nc.sync.dma_start(out=v[n], in_=_bcast_ap(vec_d[n]))
        b31 = const.tile([128, 1], F32, name="b31")  # b13 + b21
        nc.vector.tensor_tensor(
            out=b31, in0=v["b13"], in1=v["b21"], op=mybir.AluOpType.add
        )
        b32 = const.tile([128, 1], F32, name="b32")  # b13 + b22
        nc.vector.tensor_tensor(
            out=b32, in0=v["b13"], in1=v["b22"], op=mybir.AluOpType.add
        )
        # DVE signs compare against the negated bias: s = (u >= -b) - 0.5
        nb31 = const.tile([128, 1], F32, name="nb31")
        nc.vector.tensor_scalar_mul(nb31, b31, -1.0)
        nb11 = const.tile([128, 1], F32, name="nb11")
        nc.vector.tensor_scalar_mul(nb11, v["b11"], -1.0)

        w1, as1, id1 = _prep_conv_consts(nc, const, w3_d, "c1",
                                         samp=0.5 if S1_DVE else 1.0)
        w2, as2, id2 = _prep_conv_consts(nc, const, wpw_d, "c2",
                                         samp=0.5 if S2_DVE else 1.0)

        loop_cm = tc.For_i(0, loop_n, 1) if loop_n else nullcontext()
        with loop_cm:
            _main_strips(tc, nc, out_d, x_d, bl, v, b31, b32, nb11, nb31,
                         w1, as1, id1, w2, as2, id2,
                         s1pool, p1pool, s2pool, p2pool, ps)


def _main_strips(tc, nc, out_d, x_d, bl, v, b31, b32, nb11, nb31,
                 w1, as1, id1, w2, as2, id2,
                 s1pool, p1pool, s2pool, p2pool, ps):
    X_ROWS = R + 4     # x / s1 strip rows   [h0-2, h0+R+2)
    P_ROWS = R + 2     # p1 / s2 strip rows  [h0-1, h0+R+1)
    X_LEN = X_ROWS * WP
    P_LEN = P_ROWS * WP

    # groups of 4 images (quad) or 2 (pair, AB tiles only — sim harness)
    groups = []
    i = 0
    while i < bl:
        g = min(4, bl - i)
        assert g in (2, 4)
        groups.append(list(range(i, i + g)))
        i += g

    for imgs in groups:
        nquad = len(imgs) == 4
        ngr = len(imgs) // 2
        prev_p1_ts = None

        def _strip_rows(si):
            h0i = si * R
            c1s = max(h0i - 1, 0) if si == 0 else h0i + 1
            c1h = min(h0i + R + 1, H)
            lo_l = max(c1s - 1, 0) - (h0i - 2)
            hi_l = min(c1h + 1, H) - (h0i - 2)
            return lo_l, hi_l

        # x staging: UNPADDED bf16 with one lead element -- elem(r, c) =
        # 1 + r*W + c.  The cast-DMA lands as one contiguous run per
        # partition; the sign and the residual matmul read it through
        # 114-wide windows at 112-row-stride (overlapping APs): the wrapped
        # edge elements produce garbage that only ever reaches discarded
        # PSUM pad columns / re-zeroed s1 pad columns.
        XU_LEN = X_ROWS * W + 2

        def load_xb(si):
            # SWDGE cast DMA: fp32 HBM -> bf16, one 128-partition transfer
            # per image pair; emitted one strip AHEAD to hide under compute
            h0i = si * R
            lo_l, hi_l = _strip_rows(si)
            xloi = lo_l + h0i - 2
            xhii = hi_l + h0i - 2
            tiles = []
            for gi in range(ngr):
                xb_t = s1pool.tile([128, XU_LEN], BF16, tag=f"xb{gi}",
                                   name="xb")
                if not SKIP_LOAD:
                    src = x_d[
                        imgs[2 * gi] : imgs[2 * gi] + 2, :, xloi:xhii, :
                    ].rearrange("i c r w -> (i c) (r w)")
                    nc.gpsimd.dma_start(
                        out=xb_t[:, 1 + lo_l * W : 1 + hi_l * W], in_=src
                    )
                else:
                    nc.gpsimd.memset(xb_t[:, 1 + lo_l * W : 1 + hi_l * W],
                                     0.5)
                # the two border elements the overlapping windows touch
                # beyond the loaded rows
                nc.gpsimd.memset(xb_t[:, lo_l * W : lo_l * W + 1], 0.0)
                nc.gpsimd.memset(
                    xb_t[:, 1 + hi_l * W : 2 + hi_l * W], 0.0
                )
                tiles.append(xb_t)
            return tiles

        def _xwin(xb_t, rsl, r0_l, nrows):
            # overlapping window AP: row k of the result reads unpadded
            # elems [(r0_l+k)*W .. +114) = [wrap | x[r0_l+k,:] | wrap]
            v = xb_t[rsl, r0_l * W : r0_l * W + 1]
            return bass.AP(
                tensor=v.tensor, offset=v.offset,
                ap=[list(v.ap[0]), [W, nrows], [1, WP]],
            )

        def prep_s1(si, xb_ts_i):
            # s1 = sign(x + b11) as +-0.5; NON-overlapping strided rows on
            # both sides (overlapping input APs run ~15x slower on the DVE
            # -- only the PE residual uses the overlap window)
            lo_l, hi_l = _strip_rows(si)
            tiles = []
            for gi, xb_t in enumerate(xb_ts_i):
                s1_t = s1pool.tile([128, X_LEN + 4], BF16, tag=f"s1{gi}",
                                   name="s1")
                s1_r = s1_t[:, 2 : 2 + X_LEN].rearrange(
                    "p (r c) -> p r c", c=WP
                )
                xu_r = xb_t[:, 1 : 1 + X_ROWS * W].rearrange(
                    "p (r c) -> p r c", c=W
                )
                dst = s1_r[:, lo_l:hi_l, 1 : 1 + W]
                srcw = xu_r[:, lo_l:hi_l, :]
                if S1_DVE:
                    nc.vector.tensor_scalar(
                        out=dst, in0=srcw,
                        scalar1=nb11, scalar2=0.5,
                        op0=mybir.AluOpType.is_ge,
                        op1=mybir.AluOpType.subtract,
                    )
                else:
                    nc.scalar.activation(
                        out=dst, in_=srcw,
                        func=mybir.ActivationFunctionType.Sign,
                        bias=v["b11"],
                    )
                nc.gpsimd.memset(s1_r[:, :, 0:1], 0.0)
                nc.gpsimd.memset(s1_r[:, :, WP - 1 : WP], 0.0)
                nc.gpsimd.memset(s1_t[:, 0:2], 0.0)
                nc.gpsimd.memset(s1_t[:, 2 + X_LEN :], 0.0)
                if si == 0 and lo_l > 0:  # top image edge
                    nc.gpsimd.memset(s1_t[:, 2 : 2 + lo_l * WP], 0.0)
                if hi_l < X_ROWS:  # bottom image edge
                    nc.gpsimd.memset(
                        s1_t[:, 2 + hi_l * WP : 2 + X_LEN], 0.0
                    )
                tiles.append(s1_t)
            return tiles

        xb_ts = load_xb(0)
        s1_ts = prep_s1(0, xb_ts)
        for s in range(NSTRIPS):
            h0 = s * R
            c1lo, c1hi = max(h0 - 1, 0), min(h0 + R + 1, H)
            # rows computed by conv1 this strip; rows [h0-1, h0+1) are
            # carried over from the previous strip's p1 (no halo recompute)
            c1start = c1lo if s == 0 else h0 + 1

            def xloc(g):   # global row -> local row in x/s1 strip
                return g - (h0 - 2)

            def ploc(g):   # global row -> local row in p1/s2 strip
                return g - (h0 - 1)

            xb_next = load_xb(s + 1) if s + 1 < NSTRIPS else None

            # -- conv1 + fused residual/scale/bias/prelu -----------------
            # p1_AB = [p1_A(lo); p1_B(hi)]; p1_CD = [p1_D(lo); p1_C(hi)]
            # bf16: feeds Sign + the conv2 residual matmul (1 cyc/row)
            p1_ts = [
                p1pool.tile([128, P_LEN + 4], BF16, tag=f"p1{gi}", name="p1")
                for gi in range(ngr)
            ]

            # carry rows [h0-1, h0+1) of p1 from the previous strip
            if s > 0:
                for gi in range(ngr):
                    nc.vector.tensor_copy(
                        out=p1_ts[gi][:, 2 : 2 + 2 * WP],
                        in_=prev_p1_ts[gi][:, 2 + R * WP : 2 + (R + 2) * WP],
                    )

            for r0, nr in _row_chunks(c1start, c1hi, step=8):
                soff_of = (
                    lambda rsl, t, _r0=r0: 2
                    + (xloc(_r0) + t // 3 - 1) * WP + (t % 3 - 1)
                )
                r0_l = xloc(r0)

                def res1_of(gi):  # unpadded x through the overlap window
                    return lambda rsl, dr, snr, _g=gi: _xwin(
                        xb_ts[_g], rsl, r0_l + dr, snr
                    )

                o1 = slice(2 + ploc(r0) * WP, 2 + ploc(r0) * WP + nr * WP)
                quads = [
                    # imgA: aligned lo->lo (pair-tile 0 lo)
                    (LO, LO, s1_ts[0], res1_of(0)),
                    # imgB: aligned hi->hi (pair-tile 0 hi)
                    (HI, HI, s1_ts[0], res1_of(0)),
                ]
                pair_outs = [p1_ts[0][:, o1]]
                if nquad:
                    quads += [
                        # imgC: crossed lo->hi (home flips to hi for conv2)
                        (LO, HI, s1_ts[1], res1_of(1)),
                        # imgD: crossed hi->lo
                        (HI, LO, s1_ts[1], res1_of(1)),
                    ]
                    pair_outs.append(p1_ts[1][:, o1])
                _conv_quad(nc, ps, w1, id1, quads, pair_outs, soff_of, nr,
                           as1, v["b12"], v["a1"])

            # next strip's sign, pipelined: the load has been in flight
            # since the strip top, and the s2 pieces behind it on the DVE
            # queue gate on conv1 evictions anyway
            s1_next = prep_s1(s + 1, xb_next) if s + 1 < NSTRIPS else None

            # -- s2 = sign(p1 + b13 + b21), zero padding -----------------
            # signed in chunk-sized pieces so conv2 chunks can start as
            # soon as their input rows are evicted (no whole-strip barrier)
            s2_pieces = ([(ploc(h0 - 1), 2)] if s > 0 else []) + [
                (ploc(r0), nr) for r0, nr in _row_chunks(c1start, c1hi, step=8)
            ]
            s2_ts = []
            for gi, p1_t in enumerate(p1_ts):
                s2_t = s2pool.tile([128, P_LEN + 4], BF16, tag=f"s2{gi}",
                                   name="s2")
                s2_r = s2_t[:, 2 : 2 + P_LEN].rearrange(
                    "p (r c) -> p r c", c=WP
                )
                p1_r = p1_t[:, 2 : 2 + P_LEN].rearrange(
                    "p (r c) -> p r c", c=WP
                )
                for pr, pn in s2_pieces:
                    # strided (pad cols skipped): keeps the pad memsets
                    # free of WAW deps on the sign, so they never block
                    # the GpSimd queue at strip boundaries
                    if S2_DVE:
                        nc.vector.tensor_scalar(
                            out=s2_r[:, pr : pr + pn, 1 : 1 + W],
                            in0=p1_r[:, pr : pr + pn, 1 : 1 + W],
                            scalar1=nb31, scalar2=0.5,
                            op0=mybir.AluOpType.is_ge,
                            op1=mybir.AluOpType.subtract,
                        )
                    else:
                        nc.scalar.activation(
                            out=s2_r[:, pr : pr + pn, 1 : 1 + W],
                            in_=p1_r[:, pr : pr + pn, 1 : 1 + W],
                            func=mybir.ActivationFunctionType.Sign,
                            bias=b31,
                        )
                nc.gpsimd.memset(s2_r[:, :, 0:1], 0.0)
                nc.gpsimd.memset(s2_r[:, :, WP - 1 : WP], 0.0)
                nc.gpsimd.memset(s2_t[:, 0:2], 0.0)
                nc.gpsimd.memset(s2_t[:, 2 + P_LEN :], 0.0)
                if ploc(c1lo) > 0:
                    nc.gpsimd.memset(s2_t[:, 2 : 2 + ploc(c1lo) * WP], 0.0)
                if ploc(c1hi) < P_ROWS:
                    nc.gpsimd.memset(
                        s2_t[:, 2 + ploc(c1hi) * WP : 2 + P_LEN], 0.0
                    )
                s2_ts.append(s2_t)

            # -- conv2 + fused chain -------------------------------------
            # p2_AB = [A(lo); B(hi)]; p2_CD = [C(lo); D(hi)] (crossed back)
            # unpadded bf16: evictions strip pad cols; output cast to f32
            # on the host
            p2_ts = [
                p2pool.tile([128, R * W], BF16, tag=f"p2{gi}", name="p2")
                for gi in range(ngr)
            ]
            p2_rs = [t.rearrange("p (r c) -> p r c", c=W) for t in p2_ts]
            for r0, nr in _row_chunks(h0, h0 + R, step=8):
                soff_of = (
                    lambda rsl, t, _r0=r0: 2
                    + (ploc(_r0) + t // 3 - 1) * WP + (t % 3 - 1)
                )
                roff = 2 + ploc(r0) * WP

                def res2_of(gi):  # padded p1, flat slice
                    return lambda rsl, dr, snr, _g=gi: p1_ts[_g][
                        rsl, roff + dr * WP : roff + (dr + snr) * WP
                    ]

                r2 = slice(r0 - h0, r0 - h0 + nr)
                quads = [
                    (LO, LO, s2_ts[0], res2_of(0)),
                    (HI, HI, s2_ts[0], res2_of(0)),
                ]
                pair_outs = [p2_rs[0][:, r2, :]]
                if nquad:
                    quads += [
                        # imgC now lives on hi; crossed hi->lo back home
                        (HI, LO, s2_ts[1], res2_of(1)),
                        # imgD on lo; crossed lo->hi
                        (LO, HI, s2_ts[1], res2_of(1)),
                    ]
                    pair_outs.append(p2_rs[1][:, r2, :])
                _conv_quad(nc, ps, w2, id2, quads, pair_outs, soff_of, nr,
                           as2, b32, v["a2"], strip_pads=True)

                # out2 = p2 + b23, then store, per chunk: drains the strip
                # tail 8 rows earlier (one merged 128-partition HWDGE DMA
                # per pair per chunk)
                b23_eng = nc.gpsimd if B23_GPSIMD else nc.vector
                fs = slice((r0 - h0) * W, (r0 - h0 + nr) * W)
                for gi in range(ngr):
                    b23_eng.tensor_scalar_add(
                        p2_ts[gi][:, fs], p2_ts[gi][:, fs], v["b23"]
                    )
                    if not SKIP_STORE:
                        dst = out_d[
                            imgs[2 * gi] : imgs[2 * gi] + 2, :,
                            r0 : r0 + nr, :,
                        ].rearrange("i c r w -> (i c) (r w)")
                        nc.sync.dma_start(out=dst, in_=p2_ts[gi][:, fs])
            prev_p1_ts = p1_ts
            xb_ts = xb_next
            s1_ts = s1_next


_NC_CACHE = {}


def _get_program(bl=BL):
    if bl not in _NC_CACHE:
        _NC_CACHE[bl] = build_program(bl)
    return _NC_CACHE[bl]


def make_in_maps(inputs):
    x = np.ascontiguousarray(np.asarray(inputs["x"], dtype=np.float32))
    shared = {
        "w3": np.ascontiguousarray(np.asarray(inputs["w3"], np.float32)),
        "w_pw": np.ascontiguousarray(np.asarray(inputs["w_pw"], np.float32)),
    }
    for n in WVEC_NAMES:
        shared[n] = np.ascontiguousarray(np.asarray(inputs[n], np.float32))
    return [{"x": x[i * BL : (i + 1) * BL], **shared} for i in range(NCORES)]


def run(inputs, trace=False, **kwargs):
    nc = _get_program(BL)
    res = run_bass_kernel_spmd(
        nc, make_in_maps(inputs), core_ids=list(range(NCORES)), trace=trace,
        **kwargs,
    )
    out = np.concatenate(
        [np.asarray(r["out"], dtype=np.float32) for r in res.results], axis=0
    )
    return out, res


def kernel(**inputs):
    return run(inputs)[0]


def bench(inputs, iters=20, nc=None):
    """Steady-state wall-clock benchmark: sharded jit without donation,
    device-resident inputs, async dispatch of `iters` executions."""
    import time
    import jax
    from jax.sharding import Mesh, PartitionSpec, NamedSharding
    from jax.experimental.shard_map import shard_map
    from concourse import bass2jax as b2j

    b2j.install_neuronx_cc_hook()
    if nc is None:
        nc = _get_program(BL)
    in_maps = make_in_maps(inputs)

    in_names, out_names, out_avals = [], [], []
    for alloc in nc.m.functions[0].allocations:
        if not isinstance(mybir.MemoryLocationSet, type) or not isinstance(
            alloc, mybir.MemoryLocationSet
        ):
            continue
        name = alloc.memorylocations[0].name
        if alloc.kind == "ExternalInput":
            if nc.partition_id_tensor and name == nc.partition_id_tensor.name:
                continue
            in_names.append(name)
        elif alloc.kind == "ExternalOutput":
            out_names.append(name)
            out_avals.append(
                jax.core.ShapedArray(
                    tuple(alloc.tensor_shape), mybir.dt.np(alloc.dtype)
                )
            )
    n_params = len(in_names)
    all_names = in_names + out_names
    if nc.partition_id_tensor:
        all_names = all_names + [nc.partition_id_tensor.name]

    def _body(*args):
        operands = list(args)
        if nc.partition_id_tensor:
            operands.append(b2j.partition_id_tensor())
        outs = b2j._bass_exec_p.bind(
            *operands,
            out_avals=tuple(out_avals),
            in_names=tuple(all_names),
            out_names=tuple(out_names),
            lowering_input_output_aliases=(),
            sim_require_finite=True,
            sim_require_nnan=True,
            nc=nc,
        )
        return tuple(outs)

    devices = jax.devices()[:NCORES]
    mesh = Mesh(np.asarray(devices), ("core",))
    nin = n_params + len(out_names)
    f = jax.jit(
        shard_map(
            _body,
            mesh=mesh,
            in_specs=(PartitionSpec("core"),) * nin,
            out_specs=(PartitionSpec("core"),) * len(out_names),
            check_rep=False,
        ),
        keep_unused=True,
    )
    sh = NamedSharding(mesh, PartitionSpec("core"))
    concat_in = [
        jax.device_put(np.concatenate([m[n] for m in in_maps], axis=0), sh)
        for n in in_names
    ]
    zeros = [
        jax.device_put(
            np.zeros((NCORES * a.shape[0], *a.shape[1:]), a.dtype), sh
        )
        for a in out_avals
    ]

    r = f(*concat_in, *zeros)  # warm-up / compile
    jax.block_until_ready(r)

    ts = []
    for _ in range(max(iters, 8)):
        t0 = time.perf_counter()
        r = f(*concat_in, *zeros)
        jax.block_until_ready(r)
        ts.append(time.perf_counter() - t0)
    return {"single_s": min(ts), "all": ts}


def bench_device(inputs, loops=(64, 1024), calls=10):
    """Per-iteration device time via on-device For_i repetition.  The two
    loop-count programs are dispatched in interleaved alternation so slow
    drift in dispatch overhead cancels out of the slope."""
    import time
    import jax
    from jax.sharding import Mesh, PartitionSpec, NamedSharding

    fns = {}
    for L in loops:
        nc = build_program(BL, loop_n=L)
        fns[L] = _bench_fn(inputs, nc)
    ts = {L: [] for L in loops}
    for L in loops:  # warm-up / compile
        jax.block_until_ready(fns[L]())
    for _ in range(calls):
        for L in loops:
            t0 = time.perf_counter()
            jax.block_until_ready(fns[L]())
            ts[L].append(time.perf_counter() - t0)
    res = {L: min(v) for L, v in ts.items()}
    for L in loops:
        print(f"  loop_n={L}: best single call {res[L] * 1e3:.2f} ms")
    l0, l1 = loops
    per_iter = (res[l1] - res[l0]) / (l1 - l0)
    return {"per_iter_s": per_iter, "times": res}


def _bench_fn(inputs, nc):
    """Build a zero-copy dispatch closure for `nc` (device-resident args)."""
    import jax
    from jax.sharding import Mesh, PartitionSpec, NamedSharding
    from jax.experimental.shard_map import shard_map
    from concourse import bass2jax as b2j

    b2j.install_neuronx_cc_hook()
    in_maps = make_in_maps(inputs)
    in_names, out_names, out_avals = [], [], []
    for alloc in nc.m.functions[0].allocations:
        if not isinstance(alloc, mybir.MemoryLocationSet):
            continue
        name = alloc.memorylocations[0].name
        if alloc.kind == "ExternalInput":
            if nc.partition_id_tensor and name == nc.partition_id_tensor.name:
                continue
            in_names.append(name)
        elif alloc.kind == "ExternalOutput":
            out_names.append(name)
            out_avals.append(
                jax.core.ShapedArray(
                    tuple(alloc.tensor_shape), mybir.dt.np(alloc.dtype)
                )
            )
    all_names = in_names + out_names
    if nc.partition_id_tensor:
        all_names = all_names + [nc.partition_id_tensor.name]

    def _body(*args):
        operands = list(args)
        if nc.partition_id_tensor:
            operands.append(b2j.partition_id_tensor())
        return tuple(
            b2j._bass_exec_p.bind(
                *operands,
                out_avals=tuple(out_avals),
                in_names=tuple(all_names),
                out_names=tuple(out_names),
                lowering_input_output_aliases=(),
                sim_require_finite=True,
                sim_require_nnan=True,
                nc=nc,
            )
        )

    devices = jax.devices()[:NCORES]
    mesh = Mesh(np.asarray(devices), ("core",))
    nin = len(in_names) + len(out_names)
    f = jax.jit(
        shard_map(
            _body, mesh=mesh,
            in_specs=(PartitionSpec("core"),) * nin,
            out_specs=(PartitionSpec("core"),) * len(out_names),
            check_rep=False,
        ),
        keep_unused=True,
    )
    sh = NamedSharding(mesh, PartitionSpec("core"))
    concat_in = [
        jax.device_put(np.concatenate([m[n] for m in in_maps], axis=0), sh)
        for n in in_names
    ]
    zeros = [
        jax.device_put(
            np.zeros((NCORES * a.shape[0], *a.shape[1:]), a.dtype), sh
        )
        for a in out_avals
    ]
    return lambda: f(*concat_in, *zeros)


if __name__ == "__main__":
    rng = np.random.default_rng(0)
    ins = {"x": rng.standard_normal((B, C, H, W)).astype(np.float32)}
    for n in ["w3", "w_pw"]:
        ins[n] = ((rng.random((C, C, 3, 3)) - 0.5) * 0.002).astype(np.float32)
    for n in WVEC_NAMES:
        ins[n] = (rng.standard_normal(C) * 0.01).astype(np.float32)
    out = kernel(**ins)
    print(out.shape, out.dtype)



# revision 41
# speedup vs baseline: 3.3694x; 1.0608x over previous
"""Trainium2 Bass kernel for a ReActNet-style binary BasicBlock.

Reference math per block (twice, with different weights):
    s   = sign(x + b_in)                      # +-1
    c   = conv3x3(s, mean|w| * sign(w))       # binarized conv, pad=1
    y   = x + ALPHA * c                       # residual
    y   = prelu(y + b_mid, a) + b_out

Key facts exploited:
  * matmul inputs are exactly +-1 -> bf16 matmuls are EXACT (integer sums
    accumulated in fp32 PSUM).
  * per-output-channel weight scale factors out:  conv(s, scale*sign(w)) =
    scale .* conv(s, sign(w)).
  * residual rides through PSUM via a bf16 matmul with diag(1/(ALPHA*scale))
    (bf16 streams 1 cycle/row on the PE; fp32 costs 4): x and p1 are held /
    evicted as bf16 (~0.4% rounding, well inside the 2e-2 gate).
        T = binconv(s) + x / as           (as = ALPHA*scale, per channel)
    then prelu(x + as*binconv + b, a) = Prelu-activation(T) with
    per-partition scale=as, bias=b, alpha=a  -- a single ScalarE op reading
    PSUM directly.  (prelu positive homogeneity: as > 0.)

Layout (q4i scheme): NCHW, channels (64) on partitions; FOUR images in
flight on the four 64x64 PE tiles, PAIRED into two PSUM banks (diagonal
row-split pairs write complementary partition halves of one bank; the
per-partition has_written zero-regions keep the accumulation groups
independent):
    imgA: tile(0,0)   rows 0-63  -> bankX 0-63    (aligned)
    imgB: tile(64,64) rows 64-127-> bankX 64-127  (aligned)
    imgC: tile(0,64) / tile(64,0) alternating per conv -> bankY (crossed)
    imgD: tile(64,0) / tile(0,64) alternating per conv -> bankY (crossed)
The crossed tiles flip C/D's partition home after each conv; all engine ops
stay partition-aligned, DMA handles the final placement for free.
Bank pairing lets every PSUM eviction run as ONE 128-partition ScalarE
Prelu op per pair (vs two 64-partition ops): half the ACT instructions at
full lane utilization.

Engine balance: signs run on the DVE as single chained tensor_scalar ops
producing +-0.5 ((x >= -b) - 0.5); the missing x2 is folded into the
per-channel eviction scale (asc = 2*ALPHA*mean|w|), keeping matmuls
exact.  ScalarE does nothing but the fused Prelu evictions; GpSimd does
memsets + the SWDGE cast-loads; Sync issues the stores (HWDGE).
Hard-won scheduling facts baked in here: GpSimd/Pool tensor_scalar is
~15x slower than DVE; overlapping-row input APs are ~15x slower on the
DVE (but free on the PE rhs port); pad-col memsets must carry no WAW dep
on the sign ops or they head-of-line-block the GpSimd FIFO into the next
strip's conv1.

Spatial strips of R=16 output rows, processed as 8-row double-chunks.
x is cast-DMA'd (fp32->bf16, SWDGE) into contiguous unpadded staging
(one descriptor run per partition); the DVE sign restrides it to the
padded-114 conv layout, and the conv1 residual matmul streams it
directly through an overlapping 114-wide/112-stride window AP whose
wrapped edge values land in discarded PSUM pad columns.  p2 is evicted
unpadded bf16 and stored per-chunk; the f32 cast happens on the host.
conv1 rows [h0-1,h0+1) are carried over from the previous strip's p1 (no
halo recompute); s2 is signed in chunk-sized pieces so conv2 can start
before the whole strip is evicted; the next strip's s1 sign runs
mid-strip so strip boundaries carry no DVE work.

Sharding: pure data parallel, batch 32 -> 4 images x 8 cores, weights
replicated, no collectives.
"""

import numpy as np
from contextlib import ExitStack, nullcontext

import concourse.bass as bass
import concourse.tile as tile
from concourse import mybir
from concourse import bacc
from concourse.bass_utils import run_bass_kernel_spmd
from concourse.masks import make_identity

B, C, H, W = 32, 64, 112, 112
ALPHA = 0.25
NCORES = 8
BL = B // NCORES          # images per core
WP = W + 2                # padded width
R = 16                    # output rows per strip
NSTRIPS = H // R

F32 = mybir.dt.float32
BF16 = mybir.dt.bfloat16

WVEC_NAMES = ["b11", "b12", "b13", "b21", "b22", "b23", "a1", "a2"]

SKIP_STORE = False   # timing experiment: drop output DMA
SKIP_LOAD = False    # timing experiment: drop input DMA (garbage data)
RESID_LAST = True    # residual matmul in slot 9 (vs slot 0)
P2_BUFS = 2          # p2 double/triple buffering
S1_DVE = True        # s1 sign on DVE (else ScalarE Sign, +-1 w/ asc1 x1)
S2_DVE = True        # s2 sign on DVE (else ScalarE Sign)
B23_GPSIMD = False   # final +b23 on GpSimd (Pool tensor_scalar: ~15x
                     # slower than DVE -- measured 13us/op; keep False)


def _bcast_ap(dram_ap, reps=2):
    """Source AP replicating a DRAM tensor across partition groups."""
    return bass.AP(
        tensor=dram_ap.tensor,
        offset=dram_ap.offset,
        ap=[[0, reps]] + [list(d) for d in dram_ap.ap],
    )


def _row_chunks(lo, hi, step=4):
    r = lo
    while r < hi:
        yield r, min(step, hi - r)
        r += step


def build_program(bl=BL, loop_n=None, unroll=1):
    """Build the Bass program for one core processing `bl` images.

    loop_n: if set, repeat the whole main loop on-device that many times
    (timing harness only -- results identical, just recomputed).
    unroll: copies of the main loop per For_i iteration (timing only) --
    amortizes the ~9us of loop-back branch/drain/sem-reset machinery
    that is not part of a real single-shot execution.
    """
    nc = bacc.Bacc("TRN2", target_bir_lowering=False, debug=False)

    x_d = nc.dram_tensor("x", [bl, C, H, W], F32, kind="ExternalInput").ap()
    w3_d = nc.dram_tensor("w3", [C, C, 3, 3], F32, kind="ExternalInput").ap()
    wpw_d = nc.dram_tensor("w_pw", [C, C, 3, 3], F32, kind="ExternalInput").ap()
    vec_d = {
        n: nc.dram_tensor(n, [C], F32, kind="ExternalInput").ap()
        for n in WVEC_NAMES
    }
    # bf16 output (host casts back to f32): halves the store traffic and
    # stays well inside the 2e-2 gate (adds <= 2^-9 relative rounding)
    out_d = nc.dram_tensor("out", [bl, C, H, W], BF16, kind="ExternalOutput").ap()

    with tile.TileContext(nc) as tc:
        _kernel_body(tc, out_d, x_d, w3_d, wpw_d, vec_d, bl, loop_n=loop_n,
                     unroll=unroll)

    nc.compile()
    return nc


def _prep_conv_consts(nc, const, wdram, name, samp=1.0):
    """Per-conv constants: binarized-transposed weights, the eviction scale
    asc = ALPHA*mean|w|/samp (samp = sign amplitude: 0.5 when the DVE sign
    produces +-0.5), diag(1/asc) for the residual matmul.  Everything
    replicated on both partition halves."""
    # natural layout [co, ci*3*3] duplicated -> per-channel scale
    wn = const.tile([128, C * 9], F32, name=f"wn_{name}")
    nc.sync.dma_start(out=wn, in_=_bcast_ap(wdram.rearrange("a b c d -> a (b c d)")))
    wabs = const.tile([128, C * 9], F32, name=f"wabs_{name}")
    asum = const.tile([128, 1], F32, name=f"asum_{name}")
    nc.scalar.activation(
        out=wabs, in_=wn, func=mybir.ActivationFunctionType.Abs, accum_out=asum
    )
    asc = const.tile([128, 1], F32, name=f"asc_{name}")
    nc.vector.tensor_scalar_mul(asc, asum, ALPHA / (C * 9) / samp)
    inv_asc = const.tile([128, 1], F32, name=f"iasc_{name}")
    nc.vector.reciprocal(inv_asc, asc)

    # gathered+transposed weights [ci(+dup), co, tap], then binarize to bf16
    wg = const.tile([128, C, 9], F32, name=f"wg_{name}")
    src = bass.AP(
        tensor=wdram.tensor,
        offset=wdram.offset,
        ap=[[9, C], [C * 9, C], [1, 9]],
    )
    for rep in range(2):
        nc.sync.dma_start(out=wg[64 * rep : 64 * rep + 64, :, :], in_=src)
    wsign = const.tile([128, C, 9], BF16, name=f"ws_{name}")
    nc.scalar.activation(out=wsign, in_=wg, func=mybir.ActivationFunctionType.Sign)

    # residual injector: diag(1/as) bf16 (bf16 matmul = 1 cyc/row vs 4 for
    # fp32), per partition half
    ident = const.tile([128, C], BF16, name=f"id_{name}")
    make_identity(nc, ident[0:64, :])
    make_identity(nc, ident[64:128, :])
    nc.vector.tensor_scalar_mul(ident, ident, inv_asc)
    return wsign, asc, ident


LO = slice(0, 64)
HI = slice(64, 128)


def _conv_quad(nc, ps, w, ident, quads, pair_outs, soff_of, nr, asc, bias_mid,
               alpha, strip_pads=False):
    """One double-chunk (up to 8 output rows) of conv for four images on the
    four 64x64 PE tiles.

    quads: list of 4 tuples (rsl, osl, s_t, res_of):
      rsl: SBUF row half this image's data lives on (slice)
      osl: PSUM partition half this image's results land on (slice)
      s_t: sign tile; res_of(rsl, dr, snr): residual rhs AP for a sub-chunk
    Images 0,1 share a PSUM tile ([128, 1024] = 2 banks) and images 2,3 a
    second one (complementary partition halves; per-partition has_written
    keeps the accumulation groups independent, and the diagonal tile pairs
    stream concurrently).  Rows split into two <=4-row sub-chunks, one per
    bank (cols 0:456 and 512:968), so a full 8-row double-chunk evicts as
    ONE 128-partition ACT per pair reading both banks.
    Slots: the 9 taps then the residual-injector matmul.
    Eviction: out = Prelu(asc*psum + bias) per pair.
    """
    npair = (len(quads) + 1) // 2
    pts = [ps.tile([128, 1024], F32, tag="ps", name="pt") for _ in range(npair)]
    subs = [(0, min(4, nr))] + ([(4, nr - 4)] if nr > 4 else [])
    rslot = 9 if RESID_LAST else 0
    for s in range(10):
        for qi, (rsl, osl, s_t, res_of) in enumerate(quads):
            pt = pts[qi // 2]
            for dr, snr in subs:
                nn = snr * WP
                pc = 0 if dr == 0 else 512
                if s == rslot:
                    # residual slot: taps elsewhere need only s_t
                    nc.tensor.matmul(
                        pt[osl, pc : pc + nn], ident[rsl, :],
                        res_of(rsl, dr, snr),
                        start=(s == 0), stop=(s == 9), skip_group_check=True,
                    )
                else:
                    t = s - 1 if s > rslot else s
                    so = soff_of(rsl, t) + dr * WP
                    nc.tensor.matmul(
                        pt[osl, pc : pc + nn], w[rsl, :, t],
                        s_t[rsl, so : so + nn],
                        start=(s == 0), stop=(s == 9), skip_group_check=True,
                    )
    for pt, out_ap in zip(pts, pair_outs):
        # out_ap covers the pair's full nr rows: flat [128, nr*WP] (conv1,
        # padded) or [128, nr, W] (conv2, pads stripped)
        if nr == 8:  # one ACT reading both banks
            if strip_pads:
                src = bass.AP(
                    tensor=pt.tensor, offset=pt.offset + 1,
                    ap=[list(pt.ap[0]), [512, 2], [WP, 4], [1, W]],
                )
            else:
                src = bass.AP(
                    tensor=pt.tensor, offset=pt.offset,
                    ap=[list(pt.ap[0]), [512, 2], [1, 456]],
                )
            nc.scalar.activation(
                out=out_ap, in_=src,
                func=mybir.ActivationFunctionType.Prelu,
                bias=bias_mid, scale=asc, alpha=alpha,
            )
        else:
            for dr, snr in subs:
                pc = 0 if dr == 0 else 512
                src = pt[:, pc : pc + snr * WP]
                if strip_pads:
                    src = src.rearrange("p (r c) -> p r c", c=WP)[:, :, 1 : 1 + W]
                    dst = out_ap[:, dr : dr + snr, :]
                else:
                    dst = out_ap[:, dr * WP : (dr + snr) * WP]
                nc.scalar.activation(
                    out=dst, in_=src,
                    func=mybir.ActivationFunctionType.Prelu,
                    bias=bias_mid, scale=asc, alpha=alpha,
                )


def _kernel_body(tc, out_d, x_d, w3_d, wpw_d, vec_d, bl, loop_n=None,
                 unroll=1):
    nc = tc.nc
    ctx = ExitStack()
    with ctx:
        const = ctx.enter_context(tc.tile_pool(name="const", bufs=1))
        s1pool = ctx.enter_context(tc.tile_pool(name="s1pool", bufs=2))
        p1pool = ctx.enter_context(tc.tile_pool(name="p1pool", bufs=2))
        s2pool = ctx.enter_context(tc.tile_pool(name="s2pool", bufs=2))
        p2pool = ctx.enter_context(tc.tile_pool(name="p2pool", bufs=P2_BUFS))
        # [128, 1024] psum tiles = 2 banks each; 4 bufs = all 8 banks
        ps = ctx.enter_context(tc.tile_pool(name="ps", bufs=4, space="PSUM"))

        # ---- constants -------------------------------------------------
        v = {}
        for n in WVEC_NAMES:
            v[n] = const.tile([128, 1], F32, name=f"v_{n}")
            nc.sync.dma_start(out=v[n], in_=_bcast_ap(vec_d[n]))
        b31 = const.tile([128, 1], F32, name="b31")  # b13 + b21
        nc.vector.tensor_tensor(
            out=b31, in0=v["b13"], in1=v["b21"], op=mybir.AluOpType.add
        )
        b32 = const.tile([128, 1], F32, name="b32")  # b13 + b22
        nc.vector.tensor_tensor(
            out=b32, in0=v["b13"], in1=v["b22"], op=mybir.AluOpType.add
        )
        # DVE signs compare against the negated bias: s = (u >= -b) - 0.5
        nb31 = const.tile([128, 1], F32, name="nb31")
        nc.vector.tensor_scalar_mul(nb31, b31, -1.0)
        nb11 = const.tile([128, 1], F32, name="nb11")
        nc.vector.tensor_scalar_mul(nb11, v["b11"], -1.0)

        w1, as1, id1 = _prep_conv_consts(nc, const, w3_d, "c1",
                                         samp=0.5 if S1_DVE else 1.0)
        w2, as2, id2 = _prep_conv_consts(nc, const, wpw_d, "c2",
                                         samp=0.5 if S2_DVE else 1.0)

        loop_cm = tc.For_i(0, loop_n, 1) if loop_n else nullcontext()
        with loop_cm:
            for _ in range(unroll if loop_n else 1):
                _main_strips(tc, nc, out_d, x_d, bl, v, b31, b32, nb11,
                             nb31, w1, as1, id1, w2, as2, id2,
                             s1pool, p1pool, s2pool, p2pool, ps)


def _main_strips(tc, nc, out_d, x_d, bl, v, b31, b32, nb11, nb31,
                 w1, as1, id1, w2, as2, id2,
                 s1pool, p1pool, s2pool, p2pool, ps):
    X_ROWS = R + 4     # x / s1 strip rows   [h0-2, h0+R+2)
    P_ROWS = R + 2     # p1 / s2 strip rows  [h0-1, h0+R+1)
    X_LEN = X_ROWS * WP
    P_LEN = P_ROWS * WP

    # groups of 4 images (quad) or 2 (pair, AB tiles only — sim harness)
    groups = []
    i = 0
    while i < bl:
        g = min(4, bl - i)
        assert g in (2, 4)
        groups.append(list(range(i, i + g)))
        i += g

    for imgs in groups:
        nquad = len(imgs) == 4
        ngr = len(imgs) // 2
        prev_p1_ts = None

        def _strip_rows(si):
            h0i = si * R
            c1s = max(h0i - 1, 0) if si == 0 else h0i + 1
            c1h = min(h0i + R + 1, H)
            lo_l = max(c1s - 1, 0) - (h0i - 2)
            hi_l = min(c1h + 1, H) - (h0i - 2)
            return lo_l, hi_l

        # x staging: UNPADDED bf16 with one lead element -- elem(r, c) =
        # 1 + r*W + c.  The cast-DMA lands as one contiguous run per
        # partition; the sign and the residual matmul read it through
        # 114-wide windows at 112-row-stride (overlapping APs): the wrapped
        # edge elements produce garbage that only ever reaches discarded
        # PSUM pad columns / re-zeroed s1 pad columns.
        XU_LEN = X_ROWS * W + 2

        def load_xb(si):
            # SWDGE cast DMA: fp32 HBM -> bf16, one 128-partition transfer
            # per image pair; emitted one strip AHEAD to hide under compute
            h0i = si * R
            lo_l, hi_l = _strip_rows(si)
            xloi = lo_l + h0i - 2
            xhii = hi_l + h0i - 2
            tiles = []
            for gi in range(ngr):
                xb_t = s1pool.tile([128, XU_LEN], BF16, tag=f"xb{gi}",
                                   name="xb")
                if not SKIP_LOAD:
                    src = x_d[
                        imgs[2 * gi] : imgs[2 * gi] + 2, :, xloi:xhii, :
                    ].rearrange("i c r w -> (i c) (r w)")
                    nc.gpsimd.dma_start(
                        out=xb_t[:, 1 + lo_l * W : 1 + hi_l * W], in_=src
                    )
                else:
                    nc.gpsimd.memset(xb_t[:, 1 + lo_l * W : 1 + hi_l * W],
                                     0.5)
                # the two border elements the overlapping windows touch
                # beyond the loaded rows
                nc.gpsimd.memset(xb_t[:, lo_l * W : lo_l * W + 1], 0.0)
                nc.gpsimd.memset(
                    xb_t[:, 1 + hi_l * W : 2 + hi_l * W], 0.0
                )
                tiles.append(xb_t)
            return tiles

        def _xwin(xb_t, rsl, r0_l, nrows):
            # overlapping window AP: row k of the result reads unpadded
            # elems [(r0_l+k)*W .. +114) = [wrap | x[r0_l+k,:] | wrap]
            v = xb_t[rsl, r0_l * W : r0_l * W + 1]
            return bass.AP(
                tensor=v.tensor, offset=v.offset,
                ap=[list(v.ap[0]), [W, nrows], [1, WP]],
            )

        def prep_s1(si, xb_ts_i):
            # s1 = sign(x + b11) as +-0.5; NON-overlapping strided rows on
            # both sides (overlapping input APs run ~15x slower on the DVE
            # -- only the PE residual uses the overlap window)
            lo_l, hi_l = _strip_rows(si)
            tiles = []
            for gi, xb_t in enumerate(xb_ts_i):
                s1_t = s1pool.tile([128, X_LEN + 4], BF16, tag=f"s1{gi}",
                                   name="s1")
                s1_r = s1_t[:, 2 : 2 + X_LEN].rearrange(
                    "p (r c) -> p r c", c=WP
                )
                xu_r = xb_t[:, 1 : 1 + X_ROWS * W].rearrange(
                    "p (r c) -> p r c", c=W
                )
                dst = s1_r[:, lo_l:hi_l, 1 : 1 + W]
                srcw = xu_r[:, lo_l:hi_l, :]
                if S1_DVE:
                    nc.vector.tensor_scalar(
                        out=dst, in0=srcw,
                        scalar1=nb11, scalar2=0.5,
                        op0=mybir.AluOpType.is_ge,
                        op1=mybir.AluOpType.subtract,
                    )
                else:
                    nc.scalar.activation(
                        out=dst, in_=srcw,
                        func=mybir.ActivationFunctionType.Sign,
                        bias=v["b11"],
                    )
                nc.gpsimd.memset(s1_r[:, :, 0:1], 0.0)
                nc.gpsimd.memset(s1_r[:, :, WP - 1 : WP], 0.0)
                nc.gpsimd.memset(s1_t[:, 0:2], 0.0)
                nc.gpsimd.memset(s1_t[:, 2 + X_LEN :], 0.0)
                if si == 0 and lo_l > 0:  # top image edge
                    nc.gpsimd.memset(s1_t[:, 2 : 2 + lo_l * WP], 0.0)
                if hi_l < X_ROWS:  # bottom image edge
                    nc.gpsimd.memset(
                        s1_t[:, 2 + hi_l * WP : 2 + X_LEN], 0.0
                    )
                tiles.append(s1_t)
            return tiles

        xb_ts = load_xb(0)
        s1_ts = prep_s1(0, xb_ts)
        for s in range(NSTRIPS):
            h0 = s * R
            c1lo, c1hi = max(h0 - 1, 0), min(h0 + R + 1, H)
            # rows computed by conv1 this strip; rows [h0-1, h0+1) are
            # carried over from the previous strip's p1 (no halo recompute)
            c1start = c1lo if s == 0 else h0 + 1

            def xloc(g):   # global row -> local row in x/s1 strip
                return g - (h0 - 2)

            def ploc(g):   # global row -> local row in p1/s2 strip
                return g - (h0 - 1)

            xb_next = load_xb(s + 1) if s + 1 < NSTRIPS else None

            # -- conv1 + fused residual/scale/bias/prelu -----------------
            # p1_AB = [p1_A(lo); p1_B(hi)]; p1_CD = [p1_D(lo); p1_C(hi)]
            # bf16: feeds Sign + the conv2 residual matmul (1 cyc/row)
            p1_ts = [
                p1pool.tile([128, P_LEN + 4], BF16, tag=f"p1{gi}", name="p1")
                for gi in range(ngr)
            ]

            # carry rows [h0-1, h0+1) of p1 from the previous strip
            if s > 0:
                for gi in range(ngr):
                    nc.vector.tensor_copy(
                        out=p1_ts[gi][:, 2 : 2 + 2 * WP],
                        in_=prev_p1_ts[gi][:, 2 + R * WP : 2 + (R + 2) * WP],
                    )

            for r0, nr in _row_chunks(c1start, c1hi, step=8):
                soff_of = (
                    lambda rsl, t, _r0=r0: 2
                    + (xloc(_r0) + t // 3 - 1) * WP + (t % 3 - 1)
                )
                r0_l = xloc(r0)

                def res1_of(gi):  # unpadded x through the overlap window
                    return lambda rsl, dr, snr, _g=gi: _xwin(
                        xb_ts[_g], rsl, r0_l + dr, snr
                    )

                o1 = slice(2 + ploc(r0) * WP, 2 + ploc(r0) * WP + nr * WP)
                quads = [
                    # imgA: aligned lo->lo (pair-tile 0 lo)
                    (LO, LO, s1_ts[0], res1_of(0)),
                    # imgB: aligned hi->hi (pair-tile 0 hi)
                    (HI, HI, s1_ts[0], res1_of(0)),
                ]
                pair_outs = [p1_ts[0][:, o1]]
                if nquad:
                    quads += [
                        # imgC: crossed lo->hi (home flips to hi for conv2)
                        (LO, HI, s1_ts[1], res1_of(1)),
                        # imgD: crossed hi->lo
                        (HI, LO, s1_ts[1], res1_of(1)),
                    ]
                    pair_outs.append(p1_ts[1][:, o1])
                _conv_quad(nc, ps, w1, id1, quads, pair_outs, soff_of, nr,
                           as1, v["b12"], v["a1"])

            # next strip's sign, pipelined: the load has been in flight
            # since the strip top, and the s2 pieces behind it on the DVE
            # queue gate on conv1 evictions anyway
            s1_next = prep_s1(s + 1, xb_next) if s + 1 < NSTRIPS else None

            # -- s2 = sign(p1 + b13 + b21), zero padding -----------------
            # signed in chunk-sized pieces so conv2 chunks can start as
            # soon as their input rows are evicted (no whole-strip barrier)
            s2_pieces = ([(ploc(h0 - 1), 2)] if s > 0 else []) + [
                (ploc(r0), nr) for r0, nr in _row_chunks(c1start, c1hi, step=8)
            ]
            s2_ts = []
            for gi, p1_t in enumerate(p1_ts):
                s2_t = s2pool.tile([128, P_LEN + 4], BF16, tag=f"s2{gi}",
                                   name="s2")
                s2_r = s2_t[:, 2 : 2 + P_LEN].rearrange(
                    "p (r c) -> p r c", c=WP
                )
                p1_r = p1_t[:, 2 : 2 + P_LEN].rearrange(
                    "p (r c) -> p r c", c=WP
                )
                for pr, pn in s2_pieces:
                    # strided (pad cols skipped): keeps the pad memsets
                    # free of WAW deps on the sign, so they never block
                    # the GpSimd queue at strip boundaries
                    if S2_DVE:
                        nc.vector.tensor_scalar(
                            out=s2_r[:, pr : pr + pn, 1 : 1 + W],
                            in0=p1_r[:, pr : pr + pn, 1 : 1 + W],
                            scalar1=nb31, scalar2=0.5,
                            op0=mybir.AluOpType.is_ge,
                            op1=mybir.AluOpType.subtract,
                        )
                    else:
                        nc.scalar.activation(
                            out=s2_r[:, pr : pr + pn, 1 : 1 + W],
                            in_=p1_r[:, pr : pr + pn, 1 : 1 + W],
                            func=mybir.ActivationFunctionType.Sign,
                            bias=b31,
                        )
                nc.gpsimd.memset(s2_r[:, :, 0:1], 0.0)
                nc.gpsimd.memset(s2_r[:, :, WP - 1 : WP], 0.0)
                nc.gpsimd.memset(s2_t[:, 0:2], 0.0)
                nc.gpsimd.memset(s2_t[:, 2 + P_LEN :], 0.0)
                if ploc(c1lo) > 0:
                    nc.gpsimd.memset(s2_t[:, 2 : 2 + ploc(c1lo) * WP], 0.0)
                if ploc(c1hi) < P_ROWS:
                    nc.gpsimd.memset(
                        s2_t[:, 2 + ploc(c1hi) * WP : 2 + P_LEN], 0.0
                    )
                s2_ts.append(s2_t)

            # -- conv2 + fused chain -------------------------------------
            # p2_AB = [A(lo); B(hi)]; p2_CD = [C(lo); D(hi)] (crossed back)
            # unpadded bf16: evictions strip pad cols; output cast to f32
            # on the host
            p2_ts = [
                p2pool.tile([128, R * W], BF16, tag=f"p2{gi}", name="p2")
                for gi in range(ngr)
            ]
            p2_rs = [t.rearrange("p (r c) -> p r c", c=W) for t in p2_ts]
            for r0, nr in _row_chunks(h0, h0 + R, step=8):
                soff_of = (
                    lambda rsl, t, _r0=r0: 2
                    + (ploc(_r0) + t // 3 - 1) * WP + (t % 3 - 1)
                )
                roff = 2 + ploc(r0) * WP

                def res2_of(gi):  # padded p1, flat slice
                    return lambda rsl, dr, snr, _g=gi: p1_ts[_g][
                        rsl, roff + dr * WP : roff + (dr + snr) * WP
                    ]

                r2 = slice(r0 - h0, r0 - h0 + nr)
                quads = [
                    (LO, LO, s2_ts[0], res2_of(0)),
                    (HI, HI, s2_ts[0], res2_of(0)),
                ]
                pair_outs = [p2_rs[0][:, r2, :]]
                if nquad:
                    quads += [
                        # imgC now lives on hi; crossed hi->lo back home
                        (HI, LO, s2_ts[1], res2_of(1)),
                        # imgD on lo; crossed lo->hi
                        (LO, HI, s2_ts[1], res2_of(1)),
                    ]
                    pair_outs.append(p2_rs[1][:, r2, :])
                _conv_quad(nc, ps, w2, id2, quads, pair_outs, soff_of, nr,
                           as2, b32, v["a2"], strip_pads=True)

                # out2 = p2 + b23, then store, per chunk: drains the strip
                # tail 8 rows earlier (one merged 128-partition HWDGE DMA
                # per pair per chunk)
                b23_eng = nc.gpsimd if B23_GPSIMD else nc.vector
                fs = slice((r0 - h0) * W, (r0 - h0 + nr) * W)
                for gi in range(ngr):
                    b23_eng.tensor_scalar_add(
                        p2_ts[gi][:, fs], p2_ts[gi][:, fs], v["b23"]
                    )
                    if not SKIP_STORE:
                        dst = out_d[
                            imgs[2 * gi] : imgs[2 * gi] + 2, :,
                            r0 : r0 + nr, :,
                        ].rearrange("i c r w -> (i c) (r w)")
                        nc.sync.dma_start(out=dst, in_=p2_ts[gi][:, fs])
            prev_p1_ts = p1_ts
            xb_ts = xb_next
            s1_ts = s1_next


_NC_CACHE = {}


def _get_program(bl=BL):
    if bl not in _NC_CACHE:
        _NC_CACHE[bl] = build_program(bl)
    return _NC_CACHE[bl]


def make_in_maps(inputs):
    x = np.ascontiguousarray(np.asarray(inputs["x"], dtype=np.float32))
    shared = {
        "w3": np.ascontiguousarray(np.asarray(inputs["w3"], np.float32)),
        "w_pw": np.ascontiguousarray(np.asarray(inputs["w_pw"], np.float32)),
    }
    for n in WVEC_NAMES:
        shared[n] = np.ascontiguousarray(np.asarray(inputs[n], np.float32))
    return [{"x": x[i * BL : (i + 1) * BL], **shared} for i in range(NCORES)]


def run(inputs, trace=False, **kwargs):
    nc = _get_program(BL)
    res = run_bass_kernel_spmd(
        nc, make_in_maps(inputs), core_ids=list(range(NCORES)), trace=trace,
        **kwargs,
    )
    out = np.concatenate(
        [np.asarray(r["out"], dtype=np.float32) for r in res.results], axis=0
    )
    return out, res


def kernel(**inputs):
    return run(inputs)[0]


def bench(inputs, iters=20, nc=None):
    """Steady-state wall-clock benchmark: sharded jit without donation,
    device-resident inputs, async dispatch of `iters` executions."""
    import time
    import jax
    from jax.sharding import Mesh, PartitionSpec, NamedSharding
    from jax.experimental.shard_map import shard_map
    from concourse import bass2jax as b2j

    b2j.install_neuronx_cc_hook()
    if nc is None:
        nc = _get_program(BL)
    in_maps = make_in_maps(inputs)

    in_names, out_names, out_avals = [], [], []
    for alloc in nc.m.functions[0].allocations:
        if not isinstance(mybir.MemoryLocationSet, type) or not isinstance(
            alloc, mybir.MemoryLocationSet
        ):
            continue
        name = alloc.memorylocations[0].name
        if alloc.kind == "ExternalInput":
            if nc.partition_id_tensor and name == nc.partition_id_tensor.name:
                continue
            in_names.append(name)
        elif alloc.kind == "ExternalOutput":
            out_names.append(name)
            out_avals.append(
                jax.core.ShapedArray(
                    tuple(alloc.tensor_shape), mybir.dt.np(alloc.dtype)
                )
            )
    n_params = len(in_names)
    all_names = in_names + out_names
    if nc.partition_id_tensor:
        all_names = all_names + [nc.partition_id_tensor.name]

    def _body(*args):
        operands = list(args)
        if nc.partition_id_tensor:
            operands.append(b2j.partition_id_tensor())
        outs = b2j._bass_exec_p.bind(
            *operands,
            out_avals=tuple(out_avals),
            in_names=tuple(all_names),
            out_names=tuple(out_names),
            lowering_input_output_aliases=(),
            sim_require_finite=True,
            sim_require_nnan=True,
            nc=nc,
        )
        return tuple(outs)

    devices = jax.devices()[:NCORES]
    mesh = Mesh(np.asarray(devices), ("core",))
    nin = n_params + len(out_names)
    f = jax.jit(
        shard_map(
            _body,
            mesh=mesh,
            in_specs=(PartitionSpec("core"),) * nin,
            out_specs=(PartitionSpec("core"),) * len(out_names),
            check_rep=False,
        ),
        keep_unused=True,
    )
    sh = NamedSharding(mesh, PartitionSpec("core"))
    concat_in = [
        jax.device_put(np.concatenate([m[n] for m in in_maps], axis=0), sh)
        for n in in_names
    ]
    zeros = [
        jax.device_put(
            np.zeros((NCORES * a.shape[0], *a.shape[1:]), a.dtype), sh
        )
        for a in out_avals
    ]

    r = f(*concat_in, *zeros)  # warm-up / compile
    jax.block_until_ready(r)

    ts = []
    for _ in range(max(iters, 8)):
        t0 = time.perf_counter()
        r = f(*concat_in, *zeros)
        jax.block_until_ready(r)
        ts.append(time.perf_counter() - t0)
    return {"single_s": min(ts), "all": ts}


def bench_device(inputs, loops=(64, 1024), calls=10, unroll=2):
    """Per-execution device time via on-device For_i repetition (body
    unrolled x`unroll` to amortize the For_i loop-back machinery).  The
    two loop-count programs are dispatched in interleaved alternation so
    slow drift in dispatch overhead cancels out of the slope."""
    import time
    import jax
    from jax.sharding import Mesh, PartitionSpec, NamedSharding

    fns = {}
    for L in loops:
        nc = build_program(BL, loop_n=L, unroll=unroll)
        fns[L] = _bench_fn(inputs, nc)
    ts = {L: [] for L in loops}
    for L in loops:  # warm-up / compile
        jax.block_until_ready(fns[L]())
    for _ in range(calls):
        for L in loops:
            t0 = time.perf_counter()
            jax.block_until_ready(fns[L]())
            ts[L].append(time.perf_counter() - t0)
    res = {L: min(v) for L, v in ts.items()}
    for L in loops:
        print(f"  loop_n={L}: best single call {res[L] * 1e3:.2f} ms")
    l0, l1 = loops
    per_iter = (res[l1] - res[l0]) / (l1 - l0) / unroll
    return {"per_iter_s": per_iter, "times": res}


def _bench_fn(inputs, nc):
    """Build a zero-copy dispatch closure for `nc` (device-resident args)."""
    import jax
    from jax.sharding import Mesh, PartitionSpec, NamedSharding
    from jax.experimental.shard_map import shard_map
    from concourse import bass2jax as b2j

    b2j.install_neuronx_cc_hook()
    in_maps = make_in_maps(inputs)
    in_names, out_names, out_avals = [], [], []
    for alloc in nc.m.functions[0].allocations:
        if not isinstance(alloc, mybir.MemoryLocationSet):
            continue
        name = alloc.memorylocations[0].name
        if alloc.kind == "ExternalInput":
            if nc.partition_id_tensor and name == nc.partition_id_tensor.name:
                continue
            in_names.append(name)
        elif alloc.kind == "ExternalOutput":
            out_names.append(name)
            out_avals.append(
                jax.core.ShapedArray(
                    tuple(alloc.tensor_shape), mybir.dt.np(alloc.dtype)
                )
            )
    all_names = in_names + out_names
    if nc.partition_id_tensor:
        all_names = all_names + [nc.partition_id_tensor.name]

    def _body(*args):
        operands = list(args)
        if nc.partition_id_tensor:
            operands.append(b2j.partition_id_tensor())
        return tuple(
            b2j._bass_exec_p.bind(
                *operands,
                out_avals=tuple(out_avals),
                in_names=tuple(all_names),
                out_names=tuple(out_names),
                lowering_input_output_aliases=(),
                sim_require_finite=True,
                sim_require_nnan=True,
                nc=nc,
            )
        )

    devices = jax.devices()[:NCORES]
    mesh = Mesh(np.asarray(devices), ("core",))
    nin = len(in_names) + len(out_names)
    f = jax.jit(
        shard_map(
            _body, mesh=mesh,
            in_specs=(PartitionSpec("core"),) * nin,
            out_specs=(PartitionSpec("core"),) * len(out_names),
            check_rep=False,
        ),
        keep_unused=True,
    )
    sh = NamedSharding(mesh, PartitionSpec("core"))
    concat_in = [
        jax.device_put(np.concatenate([m[n] for m in in_maps], axis=0), sh)
        for n in in_names
    ]
    zeros = [
        jax.device_put(
            np.zeros((NCORES * a.shape[0], *a.shape[1:]), a.dtype), sh
        )
        for a in out_avals
    ]
    return lambda: f(*concat_in, *zeros)


if __name__ == "__main__":
    rng = np.random.default_rng(0)
    ins = {"x": rng.standard_normal((B, C, H, W)).astype(np.float32)}
    for n in ["w3", "w_pw"]:
        ins[n] = ((rng.random((C, C, 3, 3)) - 0.5) * 0.002).astype(np.float32)
    for n in WVEC_NAMES:
        ins[n] = (rng.standard_normal(C) * 0.01).astype(np.float32)
    out = kernel(**ins)
    print(out.shape, out.dtype)

